# revision 48
# baseline (speedup 1.0000x reference)
"""DigitCaps dynamic-routing kernel for 8 TRN2 NeuronCores.

Problem (hardcoded): x [256,1152,8] f32, W [1,1152,10,16,8] f32, 3 routing
iterations -> v [256,10,16,1] f32.

Strategy: shard the R=1152 routes 8-ways (144 per core), keep the full batch
B=256 on every core. u_hat is never materialized; each routing iteration
streams W through the TensorEngine:
  s_c[o,b]   = sum_{(r,i)} Ws_c[(r,i),o] * (en_c[r,b] * x[(r,i),b])   (PE)
  (AllReduce s over the 8 R-shards, squash -> v on every core)
  M_c[b,(r,i)] = sum_o v_c[b,o] * WoT_c[o,(r,i)]                      (PE)
  a_c[b,r]   = sum_i x[b,(r,i)] * M_c[b,(r,i)]                        (DVE)

Optimizations over the 373us baseline (now ~350us; ~265us of that is
kernel work after the first AllReduce - the rest is a fixed runtime
start barrier of 21+[30..55]us launch skew plus the first collective):
  - input DMAs split per-chunk across 3 engine queues so s0's first
    matmul starts as soon as chunk 0 of XT/WS lands (25us -> 13.5us);
  - s0's AllReduce is ONE merged [160,B] collective (nothing overlaps a
    split at startup); per-iteration AllReduces stay split c0..5/c6..9
    with group A triggered inside the s-phase right after c=5's
    s-matmul, so c6..9 + squash-A + a-blocks cover both collectives;
  - a "boost-warm" chain of full-width dummy matmuls after s0: the HAM
    utilization governor grants one 81920-cycle full-rate window per
    ~133us; burning a window inside the start barrier re-phases the
    governor so BOTH s-phases land in granted windows (s-matmuls at
    2 instead of 4 cycles/column);
  - the s-phase is split S1/S2: S1 (softmax + all en-transposes,
    PE-light) runs first, S2 (rep + s-matmuls, PE-dense) streams
    back-to-back into the governor's window;
  - blog is laid out [b, bh, r, c] with the capsule axis INNERMOST so
    the softmax max/sum reductions read contiguously (strided c-reads
    cost 0.56 vs 0.9 elem/ns/partition on the DVE);
  - the final s-phase transposes each s_c on the PE as it completes and
    fires per-group [b, co]-oriented AllReduces; the post-collective
    tail is just squash + the output DMA (no transposes after);
  - the a-phase runs in fp16 (WoT, x-copy, v-staging, M-matmuls, x*M
    products on the GpSimd path). Numerically safe: the a-phase only
    produces ROUTING LOGITS; the output path s = Ws@(en*x) stays f32r.
    fp16 M-matmuls halve the PE time under the throttled clock. NOTE:
    fp16/bf16 do NOT speed up DVE elementwise/reduce ops on TRN2
    (measured 1 elem/cycle/lane regardless of dtype);
  - squash runs entirely on DVE (|s| = max(s,-s); abs_max is not
    supported by codegen): the scalar-engine SQUARE/SQRT variants each
    cost a ~1.3us ACT_TABLE_LOAD on the post-collective critical path,
    and the staging copy writes fp16 v directly;
  - en = e*z normalization batched into one DVE op per batch-half;
  - softmax max-shift retained (logits reach ~+-70/round; exp would
    overflow); broadcasts use step-0 APs; smp softmax tiles are
    allocated once per iteration (per-call allocation adds a false WAR
    dependency that serializes the batch halves);
  - dummy keepalive matmul chains cover the collective windows.
Known dead ends (measured): bf16 collective payloads (5e-2 error AND a
slow CC path), bh-outer a-phase with overlapped softmax (DVE work is
the wall; reordering does not cut it), merged both-halves reduces
(gates on the slow GpSimd path), GpSimd anything-but-mult (Pool engine
rejects max/tensor_scalar, cannot read PSUM). W/x stay f32(r) on the
s-path: routing is chaotic under bf16 W/x rounding of the s
contraction (measured 5e-2 vs 2e-2 tolerance).
"""

import sys

if "/opt/trn_rl_repo" not in sys.path:
    sys.path.insert(0, "/opt/trn_rl_repo")

import numpy as np

import concourse.bass as bass
import concourse.tile as tile
from concourse import bacc, mybir
from concourse.bass_utils import run_bass_kernel_spmd
from concourse.masks import make_identity

F32 = mybir.dt.float32
F32R = mybir.dt.float32r
BF16 = mybir.dt.bfloat16
FP16 = mybir.dt.float16

NCORES = 8
B, R, C, O, I = 256, 1152, 10, 16, 8
RL = R // NCORES          # 144 routes per core
RI = RL * I               # 1152 (r,i) rows per core
NT = RI // 128            # 9 K-chunks of 128
CO = C * O                # 160
BH = B // 128             # 2 batch half-tiles
HA = RI // 3              # 384-wide a-phase chunks
CA, CB = 6, 4             # AllReduce capsule groups c0..5 / c6..9
NA, NB_ = CA * O, CB * O  # 96 / 64 bounce rows

AP = bass.AP


def _insert_bcast(base, pos, count):
    """Insert a step-0 (broadcast) free dim into an existing AP at index pos."""
    dims = list(base.ap)
    dims.insert(pos, [0, count])
    return AP(tensor=base.tensor, offset=base.offset, ap=dims)


def build_kernel(n_iters: int, reps: int = 1, collectives: bool = True):
    nc = bacc.Bacc("TRN2", target_bir_lowering=False, debug=False,
                   num_devices=NCORES)

    xt_in = nc.dram_tensor("xt", [128, NT, B], F32R, kind="ExternalInput")
    xbh_in = nc.dram_tensor("xbh", [128, BH, RI], FP16, kind="ExternalInput")
    ws = nc.dram_tensor("ws", [128, NT, CO], F32R, kind="ExternalInput")
    woth_in = nc.dram_tensor("woth", [16, C, RI], FP16, kind="ExternalInput")
    rep_in = nc.dram_tensor("rep", [128, 8, 128], BF16, kind="ExternalInput")
    rep2_in = nc.dram_tensor("rep2", [16, 128], BF16, kind="ExternalInput")
    out = nc.dram_tensor("out", [B, CO], F32, kind="ExternalOutput")

    with tile.TileContext(nc) as tc:
        with (
            tc.tile_pool(name="stat", bufs=1) as stat,
            tc.tile_pool(name="work", bufs=2) as work,
            tc.tile_pool(name="sm", bufs=1) as smp,
            tc.tile_pool(name="mtp", bufs=4) as mtp,
            tc.tile_pool(name="ytp", bufs=2) as ytp,
            tc.tile_pool(name="dram", bufs=2, space="DRAM") as dram,
            tc.tile_pool(name="ps_mp", bufs=2, space="PSUM") as ps_mp,
            tc.tile_pool(name="ps_yp", bufs=2, space="PSUM") as ps_yp,
            tc.tile_pool(name="ps_ep", bufs=2, space="PSUM") as ps_ep,
            tc.tile_pool(name="ps_sp", bufs=2, space="PSUM") as ps_sp,
        ):
            # ---- static SBUF tensors ----
            XT = stat.tile([128, NT, B], F32R)        # x^T  [(r,i)%128, t, b]
            XBH = stat.tile([128, BH, RI], FP16)      # x    [b%128, bh, (r,i)]
            WS = stat.tile([128, NT, CO], F32R)       # W as lhsT for s-matmul
            WOTH = stat.tile([16, C, RI], FP16)       # W^T as rhs for M-matmul
            REP = stat.tile([128, 8, 128], BF16)      # replicate-by-8 selectors
            REP2 = stat.tile([16, 128], BF16)         # chunk t=8 selector
            IDENT = stat.tile([128, 128], F32)
            IDENTB = stat.tile([128, 128], BF16)
            # chunked + spread over queues: s0's chunk-t matmuls start as
            # soon as their slices land instead of waiting for the whole
            # 1.9MB on one queue.
            for g in range(3):
                ts = slice(3 * g, 3 * g + 3)
                qw = (nc.gpsimd, nc.scalar, nc.gpsimd)[g]
                nc.sync.dma_start(out=XT[:, ts, :], in_=xt_in[:, ts, :])
                qw.dma_start(out=WS[:, ts, :], in_=ws[:, ts, :])
            nc.scalar.dma_start(out=XBH, in_=xbh_in[:])
            nc.gpsimd.dma_start(out=WOTH, in_=woth_in[:])
            nc.scalar.dma_start(out=REP, in_=rep_in[:])
            nc.scalar.dma_start(out=REP2, in_=rep2_in[:])
            make_identity(nc, IDENT[:, :])
            make_identity(nc, IDENTB[:, :])

            # logits b_ij, layout [p=b%128, (bh, r, c)]: c innermost so
            # the softmax max/sum reductions read contiguously
            blog = stat.tile([128, BH, RL, C], F32)

            # v (squashed capsule outputs), [co, b] layout split by
            # AllReduce group (co 0..96 / 96..160). f32 copies feed the
            # n_iters==1 output transpose; fp16 copies feed the a-phase
            # staging (the a-phase only makes routing logits).
            vA = stat.tile([NA, B], F32)
            vB = stat.tile([NB_, B], F32)
            vhA = stat.tile([NA, B], FP16)
            vhB = stat.tile([NB_, B], FP16)
            # vrt[o, c, b]: a-phase lhsT must start at partition 0,
            # so v-slices are re-staged per capsule via small SBUF DMAs.
            vrt = stat.tile([16, C, B], FP16)
            # en^T staging for the whole s-phase (S1 fills, S2 consumes)
            ET1 = stat.tile([128, C, B], BF16)
            ET2 = stat.tile([16, C, B], BF16)

            def stage_vrt(cs):
                for c in cs:
                    src = (vhA[16 * c:16 * (c + 1), :] if c < CA
                           else vhB[16 * (c - CA):16 * (c - CA + 1), :])
                    qeng = nc.sync if (c % 2 == 0) else nc.scalar
                    qeng.dma_start(out=vrt[:, c, :], in_=src)

            def allreduce_g(writes, grp):
                """One c-group's AllReduce. writes: (ptile, nrows, row0).
                grp 0 covers co 0..96 (c 0..5), grp 1 covers co 96..160.
                Returns (bounce_out, last_drain_tile)."""
                nr = NA if grp == 0 else NB_
                b_in = dram.tile([nr, B], F32, tag=f"ari{grp}")
                b_out = dram.tile([nr, B], F32, tag=f"aro{grp}")
                last_sb = None
                for ptile, nrows, r0 in writes:
                    sb = work.tile([nrows, B], F32, tag=f"sd{grp}_{nrows}")
                    nc.scalar.copy(sb[:, :], ptile[0:nrows, :])
                    nc.sync.dma_start(out=b_in[r0:r0 + nrows, :], in_=sb)
                    last_sb = sb
                if collectives:
                    nc.gpsimd.collective_compute(
                        "AllReduce",
                        mybir.AluOpType.add,
                        replica_groups=[list(range(NCORES))],
                        ins=[b_in[:].opt()],
                        outs=[b_out[:].opt()],
                    )
                else:
                    nc.sync.dma_start(out=b_out[:], in_=b_in[:])
                return b_out, last_sb

            def keepalive(seed, n, nb=128, lhs=None):
                """Chain of dummy accumulating matmuls anchored on `seed`:
                holds the PE HAM activity window open across an engine-idle
                stretch so later matmuls avoid the 1.2GHz cold clock."""
                kp = ps_sp.tile([16, nb], F32, tag="sp")
                li = lhs if lhs is not None else IDENT
                for i in range(n):
                    nc.tensor.matmul(kp, li[0:16, 0:16], seed,
                                     start=(i == 0), stop=(i == n - 1))

            def boost_warm(n):
                """Full-width dummy matmul stream: trips the HAM governor's
                full-rate window while the runtime start barrier blocks the
                first collective anyway, re-phasing the ~133us boost cadence
                so both s-phases land inside granted windows."""
                kb = ps_yp.tile([128, 2 * B], F32, tag="yp")
                for j in range(n):
                    nc.tensor.matmul(kb[:, 0:B], WS[:, 0, 0:128],
                                     XT[:, j % NT, :],
                                     start=(j == 0), stop=(j == n - 1))

            def s0_matmul():
                """s0 = sum_r u_hat -> ONE merged [160, B] AllReduce (there
                is nothing to overlap a split with at startup, and merging
                frees group B ~10us earlier)."""
                p1 = ps_ep.tile([NA, B], F32, tag="ep")
                p2 = ps_sp.tile([NB_, B], F32, tag="sp")
                for t in range(NT):
                    nc.tensor.matmul(p1, WS[:, t, 0:NA], XT[:, t, :],
                                     start=(t == 0), stop=(t == NT - 1))
                    nc.tensor.matmul(p2, WS[:, t, NA:CO], XT[:, t, :],
                                     start=(t == 0), stop=(t == NT - 1))
                boA, sdA = allreduce_g([(p1, NA, 0)], 0)
                boB, _ = allreduce_g([(p2, NB_, 0)], 1)
                boost_warm(60)
                keepalive(sdA[0:16, 0:128], 20)
                return boA, boB

            def squash_chain(sf, sq, ab, den, v, scale):
                """v = s*|s|/(1+s^2) (s*=scale), all on DVE (|s|=max(s,-s)):
                no scalar SQUARE/SQRT (each ACT function switch costs a
                ~1.3us ACT_TABLE_LOAD on the squash critical path)."""
                if scale != 1.0:
                    nc.vector.tensor_scalar_mul(sf, sf, scale)
                nc.vector.tensor_scalar_mul(ab, sf, -1.0)
                nc.vector.tensor_max(ab, ab, sf)
                nc.vector.tensor_mul(sq, sf, sf)
                nc.vector.tensor_scalar_add(den, sq, 1.0)
                nc.vector.reciprocal_approx_fast(den, den)
                nc.vector.tensor_mul(den, ab, den)
                nc.vector.tensor_mul(v, den, sf)

            def squash_g(b_out, scale, grp, stage=True, row0=0):
                """load s [rows,b] from bounce, squash -> v. When staging
                for the a-phase the final mul writes the fp16 copy
                directly (no ACT cast on the staging critical path)."""
                v, vh, nr = (vA, vhA, NA) if grp == 0 else (vB, vhB, NB_)
                s = work.tile([nr, B], F32, tag=f"sq_s{nr}")
                nc.sync.dma_start(out=s, in_=b_out[row0:row0 + nr, :])
                sq = work.tile([nr, B], F32, tag=f"sq_sq{nr}")
                ab = work.tile([nr, B], F32, tag=f"sq_ab{nr}")
                den = work.tile([nr, B], F32, tag=f"sq_den{nr}")
                squash_chain(s[:, :], sq[:, :], ab[:, :], den[:, :],
                             vh[:, :] if stage else v[:, :], scale)

            _apc = [0]

            def a_psum():
                """a-phase M-chunk psum, alternating between the mpsum
                ring and the (s-phase-idle) yp ring: 4 chunk psums in
                flight lets the PE run further ahead of the DVE/ACT/GpSimd
                consumers (the a-phase is latency-bound, not rate-bound)."""
                mpa = ps_mp.tile([128, HA], F32, tag="mpsum")
                return mpa

            def a_blocks(cs, dst):
                """dst[.,bh,c,.] = sum_i x*M, M = v_c @ WoT_c (capsule group).
                M-matmuls in fp16 (half PE cost under the throttled clock).
                dst is blog itself on the first iteration, a fresh ared
                tile afterwards. ~6 blocks multiply on the DVE straight out
                of PSUM; the rest go ACT-copy(fp16) + GpSimd-multiply."""
                for c in cs:
                    for bh in range(BH):
                        lhs = vrt[:, c, bh * 128:(bh + 1) * 128]
                        if bh == 0 and c % 3 != 0:
                            # DVE multiplies straight out of PSUM
                            mt = mtp.tile([128, RI], F32, tag="mtmp")
                            for h in range(3):
                                mp = a_psum()
                                nc.tensor.matmul(
                                    mp[:, :], lhs,
                                    WOTH[:, c, h * HA:(h + 1) * HA],
                                    start=True, stop=True)
                                nc.vector.tensor_mul(
                                    mt[:, h * HA:(h + 1) * HA], mp[:, :],
                                    XBH[:, bh, h * HA:(h + 1) * HA])
                        else:
                            # ACT drains PSUM to fp16, GpSimd multiplies
                            # (keeps the DVE free: it is the binding engine)
                            mt = mtp.tile([128, RI], FP16, tag="mtmp16")
                            ms = mtp.tile([128, RI], FP16, tag="mstage")
                            for h in range(3):
                                mp = a_psum()
                                nc.tensor.matmul(
                                    mp[:, :], lhs,
                                    WOTH[:, c, h * HA:(h + 1) * HA],
                                    start=True, stop=True)
                                nc.scalar.copy(
                                    ms[:, h * HA:(h + 1) * HA], mp[:, :])
                            nc.gpsimd.tensor_mul(mt[:, :], ms[:, :],
                                                 XBH[:, bh, :])
                        tv = mt[:, :].rearrange("p (r i) -> p r i", i=I)
                        nc.vector.tensor_reduce(dst[:, bh, :, c], tv,
                                                axis=mybir.AxisListType.X,
                                                op=mybir.AluOpType.add)

            def blog_update(ar):
                for bh in range(BH):
                    nc.vector.tensor_add(blog[:, bh, :, :], blog[:, bh, :, :],
                                         ar[:, bh, :, :])

            def softmax_s1(bh, sm_tiles):
                """softmax over capsules for one batch half + en^T
                transposes into ET1/ET2. The tiles are allocated once per
                iteration by the caller: per-call allocation would add a
                false WAR dependency serializing the two halves.
                Logits reach ~±70 per routing round (squash is
                elementwise, so v saturates to +-1/element): exp needs
                the max-shift."""
                mx, e, z, en = sm_tiles
                nc.vector.tensor_reduce(mx[:, bh, :], blog[:, bh, :, :],
                                        axis=mybir.AxisListType.X,
                                        op=mybir.AluOpType.max)
                nc.vector.tensor_sub(e[:, bh, :, :], blog[:, bh, :, :],
                                     _insert_bcast(mx[:, bh, :], 2, C))
                nc.scalar.activation(e[:, bh, :, :], e[:, bh, :, :],
                                     mybir.ActivationFunctionType.Exp)
                nc.vector.tensor_reduce(z[:, bh, :], e[:, bh, :, :],
                                        axis=mybir.AxisListType.X,
                                        op=mybir.AluOpType.add)
                nc.vector.reciprocal_approx_fast(z[:, bh, :], z[:, bh, :])
                nc.vector.tensor_mul(en[:, bh, :, :], e[:, bh, :, :],
                                     _insert_bcast(z[:, bh, :], 2, C))
                for c in range(C):
                    ept = ps_ep.tile([128, B + 128], BF16, tag="ep")
                    bs = slice(bh * 128, (bh + 1) * 128)
                    nc.tensor.matmul(ept[:, 0:128], en[:, bh, 0:128, c],
                                     IDENTB[:, :], start=True, stop=True,
                                     is_transpose=True)
                    nc.tensor.matmul(ept[0:16, B:B + 128],
                                     en[:, bh, 128:RL, c],
                                     IDENTB[:, :], start=True, stop=True,
                                     is_transpose=True)
                    nc.scalar.copy(ET1[:, c, bs], ept[:, 0:128])
                    nc.scalar.copy(ET2[:, c, bs], ept[0:16, B:B + 128])

            def s_phase(final=False):
                """S2: rep -> y -> s-matmul streamed back-to-back
                (PE-dense, lands in the HAM boost window).
                Mid-round: AllReduce group A (c0..5) fires right after
                c=5's s-matmul; c6..9 + squash-A + a-blocks overlap the
                collectives; returns (boA, boB).
                Final round: every s_c is transposed on the PE into [b, co]
                orientation as it completes, then ONE AllReduce reduces
                [128, bh*co]; the tail is just squash + output DMA."""
                # S2: rep -> y -> s-matmul, back-to-back on the PE
                writes = []
                fbs = []
                boA = sdA = None
                for c in range(C):
                    ytc = ytp.tile([128, NT, B], F32R, tag="ytc")
                    for pr in range(5):
                        t0 = 2 * pr
                        nn = 1 if pr == 4 else 2
                        yp = ps_yp.tile([128, 2 * B], F32, tag="yp")
                        for k in range(nn):
                            t = t0 + k
                            dst = yp[:, k * B:(k + 1) * B]
                            if t < 8:
                                nc.tensor.matmul(dst, REP[:, t, :],
                                                 ET1[:, c, :],
                                                 start=True, stop=True)
                            else:
                                nc.tensor.matmul(dst, REP2[:, :],
                                                 ET2[:, c, :],
                                                 start=True, stop=True)
                        nc.vector.tensor_mul(
                            ytc[:, t0:t0 + nn, :], yp[:, 0:nn * B],
                            XT[:, t0:t0 + nn, :])

                    sp = ps_sp.tile([16, B], F32, tag="sp")
                    for t in range(NT):
                        nc.tensor.matmul(sp, WS[:, t, c * 16:(c + 1) * 16],
                                         ytc[:, t, :],
                                         start=(t == 0), stop=(t == NT - 1))
                    if final:
                        # transpose s_c -> [b, c*16..] slices of vout now,
                        # while later capsules still compute
                        sb = work.tile([16, B], F32, tag="fsd")
                        nc.scalar.copy(sb[:, :], sp[0:16, :])
                        for bh in range(BH):
                            bs = slice(bh * 128, (bh + 1) * 128)
                            # reuse the (idle) a-phase psum ring: no extra
                            # PSUM banks for the transpose staging
                            tp = ps_mp.tile([128, HA], F32, tag="mpsum")
                            nc.tensor.matmul(tp[:, 0:16], sb[0:16, bs],
                                             IDENT[0:16, 0:16],
                                             start=True, stop=True,
                                             is_transpose=True)
                            nc.scalar.copy(
                                vout[:, bh, c * 16:(c + 1) * 16],
                                tp[:, 0:16])
                        if c == CA - 1 or c == C - 1:
                            # fire this c-group's [b, co] AllReduce now so
                            # the tail squash/DMA pipelines per group
                            g = 0 if c == CA - 1 else 1
                            n0, n1 = (0, NA) if g == 0 else (NA, CO)
                            fi = dram.tile([128, BH, n1 - n0], F32,
                                           tag=f"fbi{g}")
                            fo = dram.tile([128, BH, n1 - n0], F32,
                                           tag=f"fbo{g}")
                            nc.sync.dma_start(out=fi[:, :, :],
                                              in_=vout[:, :, n0:n1])
                            if collectives:
                                nc.gpsimd.collective_compute(
                                    "AllReduce",
                                    mybir.AluOpType.add,
                                    replica_groups=[list(range(NCORES))],
                                    ins=[fi[:].opt()],
                                    outs=[fo[:].opt()],
                                )
                            else:
                                nc.sync.dma_start(out=fo[:], in_=fi[:])
                            fbs.append(fo)
                    else:
                        writes.append((sp, 16, (c - CA if c >= CA else c) * 16))
                        if c == CA - 1:
                            boA, sdA = allreduce_g(writes[0:CA], 0)
                if final:
                    return fbs
                boB, _ = allreduce_g(writes[CA:C], 1)
                keepalive(sdA[0:16, 0:128], 14)
                return boA, boB

            # ---------------- routing ----------------
            vout = stat.tile([128, BH, CO], F32)
            odst = out[:].rearrange("(bh p) co -> p bh co", p=128)
            for _rep in range(reps):
                boA, boB = s0_matmul()
                rowB = 0
                scale = 0.1
                fb = None
                for it in range(1, n_iters):
                    dst = (blog if it == 1
                           else smp.tile([128, BH, RL, C], F32, tag="ared"))
                    squash_g(boA, scale, 0)
                    stage_vrt(range(0, CA))
                    a_blocks(range(0, 2), dst)
                    squash_g(boB, scale, 1, row0=rowB)
                    stage_vrt(range(CA, C))
                    a_blocks(range(2, CA), dst)
                    a_blocks(range(CA, C), dst)
                    if it != 1:
                        blog_update(dst)
                    sm_mx = smp.tile([128, BH, RL], F32, tag="mx")
                    sm_e = smp.tile([128, BH, RL, C], F32, tag="e")
                    sm_z = smp.tile([128, BH, RL], F32, tag="z")
                    sm_en = smp.tile([128, BH, RL, C], BF16, tag="en")
                    for bh in range(BH):
                        softmax_s1(bh, (sm_mx, sm_e, sm_z, sm_en))
                    scale = 1.0
                    rowB = 0
                    if it == n_iters - 1:
                        fb = s_phase(final=True)
                    else:
                        boA, boB = s_phase()
                if fb is not None:
                    # [b, co]-oriented tail per c-group: squash + DMA out
                    for g, fo in enumerate(fb):
                        n0, n1 = (0, NA) if g == 0 else (NA, CO)
                        w = n1 - n0
                        sfin = work.tile([128, BH, w], F32, tag=f"fs{g}")
                        nc.sync.dma_start(out=sfin, in_=fo[:, :, :])
                        sq = work.tile([128, BH, w], F32, tag=f"fsq{g}")
                        ab = work.tile([128, BH, w], F32, tag=f"fab{g}")
                        den = work.tile([128, BH, w], F32, tag=f"fden{g}")
                        vfin = work.tile([128, BH, w], F32, tag=f"fv{g}")
                        squash_chain(sfin[:, :, :], sq[:, :, :],
                                     ab[:, :, :], den[:, :, :],
                                     vfin[:, :, :], scale)
                        og = out[:, n0:n1].rearrange(
                            "(bh p) co -> p bh co", p=128)
                        nc.sync.dma_start(out=og, in_=vfin[:, :, :])
                else:
                    # n_iters == 1: squash s0's [co, b] groups + transpose
                    squash_g(boA, scale, 0, stage=False)
                    for bh in range(BH):
                        bs = slice(bh * 128, (bh + 1) * 128)
                        tp1 = ps_ep.tile([128, NA], F32, tag="ep")
                        nc.tensor.matmul(tp1, vA[:, bs], IDENT[0:NA, 0:NA],
                                         start=True, stop=True,
                                         is_transpose=True)
                        nc.scalar.copy(vout[:, bh, 0:NA], tp1[:, :])
                    squash_g(boB, scale, 1, stage=False, row0=rowB)
                    for bh in range(BH):
                        bs = slice(bh * 128, (bh + 1) * 128)
                        tp2 = ps_ep.tile([128, NB_], F32, tag="ep")
                        nc.tensor.matmul(tp2, vB[:, bs], IDENT[0:NB_, 0:NB_],
                                         start=True, stop=True,
                                         is_transpose=True)
                        nc.scalar.copy(vout[:, bh, NA:CO], tp2[:, :])
                    nc.sync.dma_start(out=odst, in_=vout[:, :, :])

    nc.compile()
    return nc


def prep_inputs(x: np.ndarray, W: np.ndarray):
    """Host-side layout prep. Returns per-core input dicts."""
    W = W[0]  # [R, C, O, I]
    # replicate-by-8 selector masks (shared across cores)
    from ml_dtypes import bfloat16
    rep = np.zeros((128, 8, 128), dtype=bfloat16)
    for t in range(8):
        for j in range(128):
            rep[16 * t + j // 8, t, j] = 1.0
    rep2 = np.zeros((16, 128), dtype=bfloat16)
    for j in range(128):
        rep2[j // 8, j] = 1.0
    in_maps = []
    for k in range(NCORES):
        rs = slice(k * RL, (k + 1) * RL)
        xk = np.ascontiguousarray(x[:, rs, :])      # [B, RL, I]
        wk = np.ascontiguousarray(W[rs])            # [RL, C, O, I]
        xt = np.transpose(xk, (1, 2, 0)).reshape(NT, 128, B)
        xt = np.transpose(xt, (1, 0, 2))            # [128, NT, B]
        xb = xk.reshape(BH, 128, RI)
        xb = np.transpose(xb, (1, 0, 2))            # [128, BH, RI]
        # ws[p, t, c*16+o] = W[16t + p//8, c, o, p%8]
        wsk = np.transpose(wk.reshape(NT, 16, C, O, I), (0, 1, 4, 2, 3))
        wsk = wsk.reshape(NT, 128, CO)
        wsk = np.transpose(wsk, (1, 0, 2))          # [128, NT, CO]
        # wot[o, c, r*8+i] = W[r, c, o, i]
        wotk = np.transpose(wk, (2, 1, 0, 3)).reshape(O, C, RI)
        f32 = np.float32
        in_maps.append({
            "xt": np.ascontiguousarray(xt).astype(f32),
            "xbh": np.ascontiguousarray(xb).astype(np.float16),
            "ws": np.ascontiguousarray(wsk).astype(f32),
            "woth": np.ascontiguousarray(wotk).astype(np.float16),
            "rep": rep,
            "rep2": rep2,
        })
    return in_maps


_CACHE = {}


def _get_nc(n_iters: int):
    if n_iters not in _CACHE:
        _CACHE[n_iters] = build_kernel(n_iters)
    return _CACHE[n_iters]


def kernel(x, W, num_iterations, _trace=False):
    n = int(num_iterations)
    assert n >= 1
    nc = _get_nc(n)
    in_maps = prep_inputs(np.asarray(x, dtype=np.float32),
                          np.asarray(W, dtype=np.float32))
    res = run_bass_kernel_spmd(nc, in_maps, list(range(NCORES)),
                               trace=_trace)
    v = res.results[0]["out"].reshape(B, C, O, 1).astype(np.float32)
    kernel.last_results = res
    return v


# revision 49
# speedup vs baseline: 1.0590x; 1.0590x over previous
"""DigitCaps dynamic-routing kernel for 8 TRN2 NeuronCores.

Problem (hardcoded): x [256,1152,8] f32, W [1,1152,10,16,8] f32, 3 routing
iterations -> v [256,10,16,1] f32.

Strategy: shard the R=1152 routes 8-ways (144 per core), keep the full batch
B=256 on every core. u_hat is never materialized; each routing iteration
streams W through the TensorEngine:
  s_c[o,b]   = sum_{(r,i)} Ws_c[(r,i),o] * (en_c[r,b] * x[(r,i),b])   (PE)
  (AllReduce s over the 8 R-shards, squash -> v on every core)
  M_c[b,(r,i)] = sum_o v_c[b,o] * WoT_c[o,(r,i)]                      (PE)
  a_c[b,r]   = sum_i x[b,(r,i)] * M_c[b,(r,i)]                        (DVE)

Optimizations over the 373us baseline (now ~350us; ~265us of that is
kernel work after the first AllReduce - the rest is a fixed runtime
start barrier of 21+[30..55]us launch skew plus the first collective):
  - input DMAs split per-chunk across 3 engine queues so s0's first
    matmul starts as soon as chunk 0 of XT/WS lands (25us -> 13.5us);
  - all AllReduces are split c0..5/c6..9; the per-iteration group A is
    triggered inside the s-phase right after c=5's s-matmul, so c6..9 +
    squash-A + a-blocks cover both collectives (4/6 grouping measured
    worse: the bigger B collective lands later and slows the fin tail);
  - a "boost-warm" chain of full-width dummy matmuls after s0: the HAM
    utilization governor grants one 81920-cycle full-rate window per
    ~133us; burning a window inside the start barrier re-phases the
    governor so BOTH s-phases land in granted windows (s-matmuls at
    2 instead of 4 cycles/column);
  - the s-phase is split S1/S2: S1 (softmax + all en-transposes,
    PE-light) runs first, S2 (rep + s-matmuls, PE-dense) streams
    back-to-back into the governor's window;
  - blog is laid out [b, bh, r, c] with the capsule axis INNERMOST so
    the softmax max/sum reductions read contiguously (strided c-reads
    cost 0.56 vs 0.9 elem/ns/partition on the DVE);
  - the final s-phase transposes each s_c on the PE as it completes and
    fires per-group [b, co]-oriented AllReduces; the post-collective
    tail is just squash + the output DMA (no transposes after);
  - the a-phase runs in fp16 (WoT, x-copy, v-staging, M-matmuls, x*M
    products on the GpSimd path). Numerically safe: the a-phase only
    produces ROUTING LOGITS; the output path s = Ws@(en*x) stays f32r.
    fp16 M-matmuls halve the PE time under the throttled clock. NOTE:
    fp16/bf16 do NOT speed up DVE elementwise/reduce ops on TRN2
    (measured 1 elem/cycle/lane regardless of dtype);
  - squash runs entirely on DVE (|s| = max(s,-s); abs_max is not
    supported by codegen): the scalar-engine SQUARE/SQRT variants each
    cost a ~1.3us ACT_TABLE_LOAD on the post-collective critical path,
    and the staging copy writes fp16 v directly;
  - en = e*z normalization batched into one DVE op per batch-half;
  - softmax max-shift retained (logits reach ~+-70/round; exp would
    overflow); broadcasts use step-0 APs; smp softmax tiles are
    allocated once per iteration (per-call allocation adds a false WAR
    dependency that serializes the batch halves);
  - dummy keepalive matmul chains cover the collective windows.
Known dead ends (measured): bf16 collective payloads (5e-2 error AND a
slow CC path), bh-outer a-phase with overlapped softmax (DVE work is
the wall; reordering does not cut it), merged both-halves reduces
(gates on the slow GpSimd path), GpSimd anything-but-mult (Pool engine
rejects max/tensor_scalar, cannot read PSUM). W/x stay f32(r) on the
s-path: routing is chaotic under bf16 W/x rounding of the s
contraction (measured 5e-2 vs 2e-2 tolerance).
"""

import sys

if "/opt/trn_rl_repo" not in sys.path:
    sys.path.insert(0, "/opt/trn_rl_repo")

import numpy as np

import concourse.bass as bass
import concourse.tile as tile
from concourse import bacc, mybir
from concourse.bass_utils import run_bass_kernel_spmd
from concourse.masks import make_identity

F32 = mybir.dt.float32
F32R = mybir.dt.float32r
BF16 = mybir.dt.bfloat16
FP16 = mybir.dt.float16

NCORES = 8
B, R, C, O, I = 256, 1152, 10, 16, 8
RL = R // NCORES          # 144 routes per core
RI = RL * I               # 1152 (r,i) rows per core
NT = RI // 128            # 9 K-chunks of 128
CO = C * O                # 160
BH = B // 128             # 2 batch half-tiles
HA = RI // 3              # 384-wide a-phase chunks
CA, CB = 6, 4             # AllReduce capsule groups c0..5 / c6..9
NA, NB_ = CA * O, CB * O  # 96 / 64 bounce rows

AP = bass.AP


def _insert_bcast(base, pos, count):
    """Insert a step-0 (broadcast) free dim into an existing AP at index pos."""
    dims = list(base.ap)
    dims.insert(pos, [0, count])
    return AP(tensor=base.tensor, offset=base.offset, ap=dims)


def build_kernel(n_iters: int, reps: int = 1, collectives: bool = True):
    nc = bacc.Bacc("TRN2", target_bir_lowering=False, debug=False,
                   num_devices=NCORES)

    xt_in = nc.dram_tensor("xt", [128, NT, B], F32R, kind="ExternalInput")
    xbh_in = nc.dram_tensor("xbh", [128, BH, RI], FP16, kind="ExternalInput")
    ws = nc.dram_tensor("ws", [128, NT, CO], F32R, kind="ExternalInput")
    woth_in = nc.dram_tensor("woth", [16, C, RI], FP16, kind="ExternalInput")
    rep_in = nc.dram_tensor("rep", [128, 8, 128], BF16, kind="ExternalInput")
    rep2_in = nc.dram_tensor("rep2", [16, 128], BF16, kind="ExternalInput")
    out = nc.dram_tensor("out", [B, CO], F32, kind="ExternalOutput")

    with tile.TileContext(nc) as tc:
        with (
            tc.tile_pool(name="stat", bufs=1) as stat,
            tc.tile_pool(name="work", bufs=2) as work,
            tc.tile_pool(name="sm", bufs=1) as smp,
            tc.tile_pool(name="mtp", bufs=4) as mtp,
            tc.tile_pool(name="ytp", bufs=2) as ytp,
            tc.tile_pool(name="dram", bufs=2, space="DRAM") as dram,
            tc.tile_pool(name="ps_mp", bufs=2, space="PSUM") as ps_mp,
            tc.tile_pool(name="ps_yp", bufs=2, space="PSUM") as ps_yp,
            tc.tile_pool(name="ps_ep", bufs=2, space="PSUM") as ps_ep,
            tc.tile_pool(name="ps_sp", bufs=2, space="PSUM") as ps_sp,
        ):
            # ---- static SBUF tensors ----
            XT = stat.tile([128, NT, B], F32R)        # x^T  [(r,i)%128, t, b]
            XBH = stat.tile([128, BH, RI], FP16)      # x    [b%128, bh, (r,i)]
            WS = stat.tile([128, NT, CO], F32R)       # W as lhsT for s-matmul
            WOTH = stat.tile([16, C, RI], FP16)       # W^T as rhs for M-matmul
            REP = stat.tile([128, 8, 128], BF16)      # replicate-by-8 selectors
            REP2 = stat.tile([16, 128], BF16)         # chunk t=8 selector
            IDENT = stat.tile([128, 128], F32)
            IDENTB = stat.tile([128, 128], BF16)
            # chunked + spread over queues: s0's chunk-t matmuls start as
            # soon as their slices land instead of waiting for the whole
            # 1.9MB on one queue.
            for g in range(3):
                ts = slice(3 * g, 3 * g + 3)
                qw = (nc.gpsimd, nc.scalar, nc.gpsimd)[g]
                nc.sync.dma_start(out=XT[:, ts, :], in_=xt_in[:, ts, :])
                qw.dma_start(out=WS[:, ts, :], in_=ws[:, ts, :])
            nc.scalar.dma_start(out=XBH, in_=xbh_in[:])
            nc.gpsimd.dma_start(out=WOTH, in_=woth_in[:])
            nc.scalar.dma_start(out=REP, in_=rep_in[:])
            nc.scalar.dma_start(out=REP2, in_=rep2_in[:])
            make_identity(nc, IDENT[:, :])
            make_identity(nc, IDENTB[:, :])

            # logits b_ij, layout [p=b%128, (bh, r, c)]: c innermost so
            # the softmax max/sum reductions read contiguously
            blog = stat.tile([128, BH, RL, C], F32)

            # v (squashed capsule outputs), [co, b] layout split by
            # AllReduce group (co 0..96 / 96..160). f32 copies feed the
            # n_iters==1 output transpose; fp16 copies feed the a-phase
            # staging (the a-phase only makes routing logits).
            vA = stat.tile([NA, B], F32)
            vB = stat.tile([NB_, B], F32)
            vhA = stat.tile([NA, B], FP16)
            vhB = stat.tile([NB_, B], FP16)
            # vrt[o, c, b]: a-phase lhsT must start at partition 0,
            # so v-slices are re-staged per capsule via small SBUF DMAs.
            vrt = stat.tile([16, C, B], FP16)
            # en^T staging for the whole s-phase (S1 fills, S2 consumes)
            ET1 = stat.tile([128, C, B], BF16)
            ET2 = stat.tile([16, C, B], BF16)

            def stage_vrt(cs):
                for c in cs:
                    src = (vhA[16 * c:16 * (c + 1), :] if c < CA
                           else vhB[16 * (c - CA):16 * (c - CA + 1), :])
                    qeng = nc.sync if (c % 2 == 0) else nc.scalar
                    qeng.dma_start(out=vrt[:, c, :], in_=src)

            def allreduce_g(writes, grp):
                """One c-group's AllReduce. writes: (ptile, nrows, row0).
                grp 0 covers co 0..96 (c 0..5), grp 1 covers co 96..160.
                Returns (bounce_out, last_drain_tile)."""
                nr = NA if grp == 0 else NB_
                b_in = dram.tile([nr, B], F32, tag=f"ari{grp}")
                b_out = dram.tile([nr, B], F32, tag=f"aro{grp}")
                last_sb = None
                for ptile, nrows, r0 in writes:
                    sb = work.tile([nrows, B], F32, tag=f"sd{grp}_{nrows}")
                    nc.scalar.copy(sb[:, :], ptile[0:nrows, :])
                    nc.sync.dma_start(out=b_in[r0:r0 + nrows, :], in_=sb)
                    last_sb = sb
                if collectives:
                    nc.gpsimd.collective_compute(
                        "AllReduce",
                        mybir.AluOpType.add,
                        replica_groups=[list(range(NCORES))],
                        ins=[b_in[:].opt()],
                        outs=[b_out[:].opt()],
                    )
                else:
                    nc.sync.dma_start(out=b_out[:], in_=b_in[:])
                return b_out, last_sb

            def keepalive(seed, n, nb=128, lhs=None):
                """Chain of dummy accumulating matmuls anchored on `seed`:
                holds the PE HAM activity window open across an engine-idle
                stretch so later matmuls avoid the 1.2GHz cold clock."""
                kp = ps_sp.tile([16, nb], F32, tag="sp")
                li = lhs if lhs is not None else IDENT
                for i in range(n):
                    nc.tensor.matmul(kp, li[0:16, 0:16], seed,
                                     start=(i == 0), stop=(i == n - 1))

            def boost_warm(n):
                """Full-width dummy matmul stream: trips the HAM governor's
                full-rate window while the runtime start barrier blocks the
                first collective anyway, re-phasing the ~133us boost cadence
                so both s-phases land inside granted windows."""
                kb = ps_yp.tile([128, 2 * B], F32, tag="yp")
                for j in range(n):
                    nc.tensor.matmul(kb[:, 0:B], WS[:, 0, 0:128],
                                     XT[:, j % NT, :],
                                     start=(j == 0), stop=(j == n - 1))

            def s0_matmul():
                """s0 = sum_r u_hat -> ONE merged [160, B] AllReduce (there
                is nothing to overlap a split with at startup, and merging
                frees group B ~10us earlier)."""
                p1 = ps_ep.tile([NA, B], F32, tag="ep")
                p2 = ps_sp.tile([NB_, B], F32, tag="sp")
                for t in range(NT):
                    nc.tensor.matmul(p1, WS[:, t, 0:NA], XT[:, t, :],
                                     start=(t == 0), stop=(t == NT - 1))
                    nc.tensor.matmul(p2, WS[:, t, NA:CO], XT[:, t, :],
                                     start=(t == 0), stop=(t == NT - 1))
                boA, sdA = allreduce_g([(p1, NA, 0)], 0)
                boB, _ = allreduce_g([(p2, NB_, 0)], 1)
                boost_warm(60)
                keepalive(sdA[0:16, 0:128], 20)
                return boA, boB

            def squash_chain(sf, sq, ab, den, v, scale):
                """v = s*|s|/(1+s^2) (s*=scale), all on DVE (|s|=max(s,-s)):
                no scalar SQUARE/SQRT (each ACT function switch costs a
                ~1.3us ACT_TABLE_LOAD on the squash critical path)."""
                if scale != 1.0:
                    nc.vector.tensor_scalar_mul(sf, sf, scale)
                nc.vector.tensor_scalar_mul(ab, sf, -1.0)
                nc.vector.tensor_max(ab, ab, sf)
                nc.vector.tensor_mul(sq, sf, sf)
                nc.vector.tensor_scalar_add(den, sq, 1.0)
                nc.vector.reciprocal_approx_fast(den, den)
                nc.vector.tensor_mul(den, ab, den)
                nc.vector.tensor_mul(v, den, sf)

            def squash_g(b_out, scale, grp, stage=True, row0=0):
                """load s [rows,b] from bounce, squash -> v. When staging
                for the a-phase the final mul writes the fp16 copy
                directly (no ACT cast on the staging critical path)."""
                v, vh, nr = (vA, vhA, NA) if grp == 0 else (vB, vhB, NB_)
                s = work.tile([nr, B], F32, tag=f"sq_s{nr}")
                nc.sync.dma_start(out=s, in_=b_out[row0:row0 + nr, :])
                sq = work.tile([nr, B], F32, tag=f"sq_sq{nr}")
                ab = work.tile([nr, B], F32, tag=f"sq_ab{nr}")
                den = work.tile([nr, B], F32, tag=f"sq_den{nr}")
                squash_chain(s[:, :], sq[:, :], ab[:, :], den[:, :],
                             vh[:, :] if stage else v[:, :], scale)

            _apc = [0]

            def a_psum():
                """a-phase M-chunk psum, alternating between the mpsum
                ring and the (s-phase-idle) yp ring: 4 chunk psums in
                flight lets the PE run further ahead of the DVE/ACT/GpSimd
                consumers (the a-phase is latency-bound, not rate-bound)."""
                mpa = ps_mp.tile([128, HA], F32, tag="mpsum")
                return mpa

            def a_blocks(cs, dst):
                """dst[.,bh,c,.] = sum_i x*M, M = v_c @ WoT_c (capsule group).
                M-matmuls in fp16 (half PE cost under the throttled clock).
                dst is blog itself on the first iteration, a fresh ared
                tile afterwards. ~6 blocks multiply on the DVE straight out
                of PSUM; the rest go ACT-copy(fp16) + GpSimd-multiply."""
                for c in cs:
                    for bh in range(BH):
                        lhs = vrt[:, c, bh * 128:(bh + 1) * 128]
                        if bh == 0 and c % 3 != 0:
                            # DVE multiplies straight out of PSUM
                            mt = mtp.tile([128, RI], F32, tag="mtmp")
                            for h in range(3):
                                mp = a_psum()
                                nc.tensor.matmul(
                                    mp[:, :], lhs,
                                    WOTH[:, c, h * HA:(h + 1) * HA],
                                    start=True, stop=True)
                                nc.vector.tensor_mul(
                                    mt[:, h * HA:(h + 1) * HA], mp[:, :],
                                    XBH[:, bh, h * HA:(h + 1) * HA])
                        else:
                            # ACT drains PSUM to fp16, GpSimd multiplies
                            # (keeps the DVE free: it is the binding engine)
                            mt = mtp.tile([128, RI], FP16, tag="mtmp16")
                            ms = mtp.tile([128, RI], FP16, tag="mstage")
                            for h in range(3):
                                mp = a_psum()
                                nc.tensor.matmul(
                                    mp[:, :], lhs,
                                    WOTH[:, c, h * HA:(h + 1) * HA],
                                    start=True, stop=True)
                                nc.scalar.copy(
                                    ms[:, h * HA:(h + 1) * HA], mp[:, :])
                            nc.gpsimd.tensor_mul(mt[:, :], ms[:, :],
                                                 XBH[:, bh, :])
                        tv = mt[:, :].rearrange("p (r i) -> p r i", i=I)
                        nc.vector.tensor_reduce(dst[:, bh, :, c], tv,
                                                axis=mybir.AxisListType.X,
                                                op=mybir.AluOpType.add)

            def blog_update(ar):
                for bh in range(BH):
                    nc.vector.tensor_add(blog[:, bh, :, :], blog[:, bh, :, :],
                                         ar[:, bh, :, :])

            def softmax_s1(bh, sm_tiles):
                """softmax over capsules for one batch half + en^T
                transposes into ET1/ET2. The tiles are allocated once per
                iteration by the caller: per-call allocation would add a
                false WAR dependency serializing the two halves.
                Logits reach ~±70 per routing round (squash is
                elementwise, so v saturates to +-1/element): exp needs
                the max-shift."""
                mx, e, z, en = sm_tiles
                nc.vector.tensor_reduce(mx[:, bh, :], blog[:, bh, :, :],
                                        axis=mybir.AxisListType.X,
                                        op=mybir.AluOpType.max)
                nc.vector.tensor_sub(e[:, bh, :, :], blog[:, bh, :, :],
                                     _insert_bcast(mx[:, bh, :], 2, C))
                nc.scalar.activation(e[:, bh, :, :], e[:, bh, :, :],
                                     mybir.ActivationFunctionType.Exp)
                nc.vector.tensor_reduce(z[:, bh, :], e[:, bh, :, :],
                                        axis=mybir.AxisListType.X,
                                        op=mybir.AluOpType.add)
                nc.vector.reciprocal_approx_fast(z[:, bh, :], z[:, bh, :])
                nc.vector.tensor_mul(en[:, bh, :, :], e[:, bh, :, :],
                                     _insert_bcast(z[:, bh, :], 2, C))
                for c in range(C):
                    ept = ps_ep.tile([128, B + 128], BF16, tag="ep")
                    bs = slice(bh * 128, (bh + 1) * 128)
                    nc.tensor.matmul(ept[:, 0:128], en[:, bh, 0:128, c],
                                     IDENTB[:, :], start=True, stop=True,
                                     is_transpose=True)
                    nc.tensor.matmul(ept[0:16, B:B + 128],
                                     en[:, bh, 128:RL, c],
                                     IDENTB[:, :], start=True, stop=True,
                                     is_transpose=True)
                    nc.scalar.copy(ET1[:, c, bs], ept[:, 0:128])
                    nc.scalar.copy(ET2[:, c, bs], ept[0:16, B:B + 128])

            def s_phase(final=False):
                """S2: rep -> y -> s-matmul streamed back-to-back
                (PE-dense, lands in the HAM boost window).
                Mid-round: AllReduce group A (c0..5) fires right after
                c=5's s-matmul; c6..9 + squash-A + a-blocks overlap the
                collectives; returns (boA, boB).
                Final round: every s_c is transposed on the PE into [b, co]
                orientation as it completes, then ONE AllReduce reduces
                [128, bh*co]; the tail is just squash + output DMA."""
                # S2: rep -> y -> s-matmul, back-to-back on the PE
                writes = []
                fbs = []
                boA = sdA = None
                for c in range(C):
                    ytc = ytp.tile([128, NT, B], F32R, tag="ytc")
                    for pr in range(5):
                        t0 = 2 * pr
                        nn = 1 if pr == 4 else 2
                        yp = ps_yp.tile([128, 2 * B], F32, tag="yp")
                        for k in range(nn):
                            t = t0 + k
                            dst = yp[:, k * B:(k + 1) * B]
                            if t < 8:
                                nc.tensor.matmul(dst, REP[:, t, :],
                                                 ET1[:, c, :],
                                                 start=True, stop=True)
                            else:
                                nc.tensor.matmul(dst, REP2[:, :],
                                                 ET2[:, c, :],
                                                 start=True, stop=True)
                        nc.vector.tensor_mul(
                            ytc[:, t0:t0 + nn, :], yp[:, 0:nn * B],
                            XT[:, t0:t0 + nn, :])

                    sp = ps_sp.tile([16, B], F32, tag="sp")
                    for t in range(NT):
                        nc.tensor.matmul(sp, WS[:, t, c * 16:(c + 1) * 16],
                                         ytc[:, t, :],
                                         start=(t == 0), stop=(t == NT - 1))
                    if final:
                        # transpose s_c -> [b, c*16..] slices of vout now,
                        # while later capsules still compute
                        sb = work.tile([16, B], F32, tag="fsd")
                        nc.scalar.copy(sb[:, :], sp[0:16, :])
                        for bh in range(BH):
                            bs = slice(bh * 128, (bh + 1) * 128)
                            # reuse the (idle) a-phase psum ring: no extra
                            # PSUM banks for the transpose staging
                            tp = ps_mp.tile([128, HA], F32, tag="mpsum")
                            nc.tensor.matmul(tp[:, 0:16], sb[0:16, bs],
                                             IDENT[0:16, 0:16],
                                             start=True, stop=True,
                                             is_transpose=True)
                            nc.scalar.copy(
                                vout[:, bh, c * 16:(c + 1) * 16],
                                tp[:, 0:16])
                        if c == CA - 1 or c == C - 1:
                            # fire this c-group's [b, co] AllReduce now so
                            # the tail squash/DMA pipelines per group
                            g = 0 if c == CA - 1 else 1
                            n0, n1 = (0, NA) if g == 0 else (NA, CO)
                            fi = dram.tile([128, BH, n1 - n0], F32,
                                           tag=f"fbi{g}")
                            fo = dram.tile([128, BH, n1 - n0], F32,
                                           tag=f"fbo{g}")
                            nc.sync.dma_start(out=fi[:, :, :],
                                              in_=vout[:, :, n0:n1])
                            if collectives:
                                nc.gpsimd.collective_compute(
                                    "AllReduce",
                                    mybir.AluOpType.add,
                                    replica_groups=[list(range(NCORES))],
                                    ins=[fi[:].opt()],
                                    outs=[fo[:].opt()],
                                )
                            else:
                                nc.sync.dma_start(out=fo[:], in_=fi[:])
                            fbs.append(fo)
                    else:
                        writes.append((sp, 16, (c - CA if c >= CA else c) * 16))
                        if c == CA - 1:
                            boA, sdA = allreduce_g(writes[0:CA], 0)
                if final:
                    return fbs
                boB, _ = allreduce_g(writes[CA:C], 1)
                keepalive(sdA[0:16, 0:128], 14)
                return boA, boB

            # ---------------- routing ----------------
            vout = stat.tile([128, BH, CO], F32)
            odst = out[:].rearrange("(bh p) co -> p bh co", p=128)
            for _rep in range(reps):
                boA, boB = s0_matmul()
                rowB = 0
                scale = 0.1
                fb = None
                for it in range(1, n_iters):
                    dst = (blog if it == 1
                           else smp.tile([128, BH, RL, C], F32, tag="ared"))
                    squash_g(boA, scale, 0)
                    stage_vrt(range(0, CA))
                    a_blocks(range(0, 2), dst)
                    squash_g(boB, scale, 1, row0=rowB)
                    stage_vrt(range(CA, C))
                    a_blocks(range(2, CA), dst)
                    a_blocks(range(CA, C), dst)
                    if it != 1:
                        blog_update(dst)
                    sm_mx = smp.tile([128, BH, RL], F32, tag="mx")
                    sm_e = smp.tile([128, BH, RL, C], F32, tag="e")
                    sm_z = smp.tile([128, BH, RL], F32, tag="z")
                    sm_en = smp.tile([128, BH, RL, C], BF16, tag="en")
                    for bh in range(BH):
                        softmax_s1(bh, (sm_mx, sm_e, sm_z, sm_en))
                    scale = 1.0
                    rowB = 0
                    if it == n_iters - 1:
                        fb = s_phase(final=True)
                    else:
                        boA, boB = s_phase()
                if fb is not None:
                    # [b, co]-oriented tail per c-group: squash + DMA out
                    for g, fo in enumerate(fb):
                        n0, n1 = (0, NA) if g == 0 else (NA, CO)
                        w = n1 - n0
                        sfin = work.tile([128, BH, w], F32, tag=f"fs{g}")
                        nc.sync.dma_start(out=sfin, in_=fo[:, :, :])
                        sq = work.tile([128, BH, w], F32, tag=f"fsq{g}")
                        ab = work.tile([128, BH, w], F32, tag=f"fab{g}")
                        den = work.tile([128, BH, w], F32, tag=f"fden{g}")
                        vfin = work.tile([128, BH, w], F32, tag=f"fv{g}")
                        squash_chain(sfin[:, :, :], sq[:, :, :],
                                     ab[:, :, :], den[:, :, :],
                                     vfin[:, :, :], scale)
                        og = out[:, n0:n1].rearrange(
                            "(bh p) co -> p bh co", p=128)
                        nc.sync.dma_start(out=og, in_=vfin[:, :, :])
                else:
                    # n_iters == 1: squash s0's [co, b] groups + transpose
                    squash_g(boA, scale, 0, stage=False)
                    for bh in range(BH):
                        bs = slice(bh * 128, (bh + 1) * 128)
                        tp1 = ps_ep.tile([128, NA], F32, tag="ep")
                        nc.tensor.matmul(tp1, vA[:, bs], IDENT[0:NA, 0:NA],
                                         start=True, stop=True,
                                         is_transpose=True)
                        nc.scalar.copy(vout[:, bh, 0:NA], tp1[:, :])
                    squash_g(boB, scale, 1, stage=False, row0=rowB)
                    for bh in range(BH):
                        bs = slice(bh * 128, (bh + 1) * 128)
                        tp2 = ps_ep.tile([128, NB_], F32, tag="ep")
                        nc.tensor.matmul(tp2, vB[:, bs], IDENT[0:NB_, 0:NB_],
                                         start=True, stop=True,
                                         is_transpose=True)
                        nc.scalar.copy(vout[:, bh, NA:CO], tp2[:, :])
                    nc.sync.dma_start(out=odst, in_=vout[:, :, :])

    nc.compile()
    return nc


def prep_inputs(x: np.ndarray, W: np.ndarray):
    """Host-side layout prep. Returns per-core input dicts."""
    W = W[0]  # [R, C, O, I]
    # replicate-by-8 selector masks (shared across cores)
    from ml_dtypes import bfloat16
    rep = np.zeros((128, 8, 128), dtype=bfloat16)
    for t in range(8):
        for j in range(128):
            rep[16 * t + j // 8, t, j] = 1.0
    rep2 = np.zeros((16, 128), dtype=bfloat16)
    for j in range(128):
        rep2[j // 8, j] = 1.0
    in_maps = []
    for k in range(NCORES):
        rs = slice(k * RL, (k + 1) * RL)
        xk = np.ascontiguousarray(x[:, rs, :])      # [B, RL, I]
        wk = np.ascontiguousarray(W[rs])            # [RL, C, O, I]
        xt = np.transpose(xk, (1, 2, 0)).reshape(NT, 128, B)
        xt = np.transpose(xt, (1, 0, 2))            # [128, NT, B]
        xb = xk.reshape(BH, 128, RI)
        xb = np.transpose(xb, (1, 0, 2))            # [128, BH, RI]
        # ws[p, t, c*16+o] = W[16t + p//8, c, o, p%8]
        wsk = np.transpose(wk.reshape(NT, 16, C, O, I), (0, 1, 4, 2, 3))
        wsk = wsk.reshape(NT, 128, CO)
        wsk = np.transpose(wsk, (1, 0, 2))          # [128, NT, CO]
        # wot[o, c, r*8+i] = W[r, c, o, i]
        wotk = np.transpose(wk, (2, 1, 0, 3)).reshape(O, C, RI)
        f32 = np.float32
        in_maps.append({
            "xt": np.ascontiguousarray(xt).astype(f32),
            "xbh": np.ascontiguousarray(xb).astype(np.float16),
            "ws": np.ascontiguousarray(wsk).astype(f32),
            "woth": np.ascontiguousarray(wotk).astype(np.float16),
            "rep": rep,
            "rep2": rep2,
        })
    return in_maps


_CACHE = {}


def _get_nc(n_iters: int):
    if n_iters not in _CACHE:
        _CACHE[n_iters] = build_kernel(n_iters)
    return _CACHE[n_iters]


def kernel(x, W, num_iterations, _trace=False):
    n = int(num_iterations)
    assert n >= 1
    nc = _get_nc(n)
    in_maps = prep_inputs(np.asarray(x, dtype=np.float32),
                          np.asarray(W, dtype=np.float32))
    res = run_bass_kernel_spmd(nc, in_maps, list(range(NCORES)),
                               trace=_trace)
    v = res.results[0]["out"].reshape(B, C, O, 1).astype(np.float32)
    kernel.last_results = res
    return v


# revision 51
# speedup vs baseline: 1.0904x; 1.0296x over previous
"""DigitCaps dynamic-routing kernel for 8 TRN2 NeuronCores.

Problem (hardcoded): x [256,1152,8] f32, W [1,1152,10,16,8] f32, 3 routing
iterations -> v [256,10,16,1] f32.

Strategy: shard the R=1152 routes 8-ways (144 per core), keep the full batch
B=256 on every core. u_hat is never materialized; each routing iteration
streams W through the TensorEngine:
  s_c[o,b]   = sum_{(r,i)} Ws_c[(r,i),o] * (en_c[r,b] * x[(r,i),b])   (PE)
  (AllReduce s over the 8 R-shards, squash -> v on every core)
  M_c[b,(r,i)] = sum_o v_c[b,o] * WoT_c[o,(r,i)]                      (PE)
  a_c[b,r]   = sum_i x[b,(r,i)] * M_c[b,(r,i)]                        (DVE)

Optimizations over the 373us baseline (now ~350us; ~265us of that is
kernel work after the first AllReduce - the rest is a fixed runtime
start barrier of 21+[30..55]us launch skew plus the first collective):
  - input DMAs split per-chunk across 3 engine queues so s0's first
    matmul starts as soon as chunk 0 of XT/WS lands (25us -> 13.5us);
  - all AllReduces are split c0..5/c6..9; the per-iteration group A is
    triggered inside the s-phase right after c=5's s-matmul, so c6..9 +
    squash-A + a-blocks cover both collectives (4/6 grouping measured
    worse: the bigger B collective lands later and slows the fin tail);
  - a "boost-warm" chain of full-width dummy matmuls after s0: the HAM
    utilization governor grants one 81920-cycle full-rate window per
    ~133us; burning a window inside the start barrier re-phases the
    governor so BOTH s-phases land in granted windows (s-matmuls at
    2 instead of 4 cycles/column);
  - the s-phase is split S1/S2: S1 (softmax + all en-transposes,
    PE-light) runs first, S2 (rep + s-matmuls, PE-dense) streams
    back-to-back into the governor's window;
  - blog is laid out [b, bh, r, c] with the capsule axis INNERMOST so
    the softmax max/sum reductions read contiguously (strided c-reads
    cost 0.56 vs 0.9 elem/ns/partition on the DVE);
  - the final s-phase transposes each s_c on the PE as it completes and
    fires per-group [b, co]-oriented AllReduces; the post-collective
    tail is just squash + the output DMA (no transposes after);
  - the a-phase runs in fp16 (WoT, x-copy, v-staging, M-matmuls, x*M
    products on the GpSimd path). Numerically safe: the a-phase only
    produces ROUTING LOGITS; the output path s = Ws@(en*x) stays f32r.
    fp16 M-matmuls halve the PE time under the throttled clock. NOTE:
    fp16/bf16 do NOT speed up DVE elementwise/reduce ops on TRN2
    (measured 1 elem/cycle/lane regardless of dtype);
  - squash runs entirely on DVE (|s| = max(s,-s); abs_max is not
    supported by codegen): the scalar-engine SQUARE/SQRT variants each
    cost a ~1.3us ACT_TABLE_LOAD on the post-collective critical path,
    and the staging copy writes fp16 v directly;
  - en = e*z normalization batched into one DVE op per batch-half;
  - softmax max-shift retained (logits reach ~+-70/round; exp would
    overflow); broadcasts use step-0 APs; smp softmax tiles are
    allocated once per iteration (per-call allocation adds a false WAR
    dependency that serializes the batch halves);
  - dummy keepalive matmul chains cover the collective windows.
Known dead ends (measured): bf16 collective payloads (5e-2 error AND a
slow CC path), bh-outer a-phase with overlapped softmax (DVE work is
the wall; reordering does not cut it), merged both-halves reduces
(gates on the slow GpSimd path), GpSimd anything-but-mult (Pool engine
rejects max/tensor_scalar, cannot read PSUM). W/x stay f32(r) on the
s-path: routing is chaotic under bf16 W/x rounding of the s
contraction (measured 5e-2 vs 2e-2 tolerance).
"""

import sys

if "/opt/trn_rl_repo" not in sys.path:
    sys.path.insert(0, "/opt/trn_rl_repo")

import numpy as np

import concourse.bass as bass
import concourse.tile as tile
from concourse import bacc, mybir
from concourse.bass_utils import run_bass_kernel_spmd
from concourse.masks import make_identity

F32 = mybir.dt.float32
F32R = mybir.dt.float32r
BF16 = mybir.dt.bfloat16
FP16 = mybir.dt.float16

NCORES = 8
B, R, C, O, I = 256, 1152, 10, 16, 8
RL = R // NCORES          # 144 routes per core
RI = RL * I               # 1152 (r,i) rows per core
NT = RI // 128            # 9 K-chunks of 128
CO = C * O                # 160
BH = B // 128             # 2 batch half-tiles
HA = RI // 3              # 384-wide a-phase chunks
CA, CB = 6, 4             # AllReduce capsule groups c0..5 / c6..9
NA, NB_ = CA * O, CB * O  # 96 / 64 bounce rows

AP = bass.AP


def _insert_bcast(base, pos, count):
    """Insert a step-0 (broadcast) free dim into an existing AP at index pos."""
    dims = list(base.ap)
    dims.insert(pos, [0, count])
    return AP(tensor=base.tensor, offset=base.offset, ap=dims)


def build_kernel(n_iters: int, reps: int = 1, collectives: bool = True):
    nc = bacc.Bacc("TRN2", target_bir_lowering=False, debug=False,
                   num_devices=NCORES)

    xt_in = nc.dram_tensor("xt", [128, NT, B], F32R, kind="ExternalInput")
    xbh_in = nc.dram_tensor("xbh", [128, BH, RI], FP16, kind="ExternalInput")
    ws = nc.dram_tensor("ws", [128, NT, CO], F32R, kind="ExternalInput")
    woth_in = nc.dram_tensor("woth", [16, C, RI], FP16, kind="ExternalInput")
    rep_in = nc.dram_tensor("rep", [128, 8, 128], BF16, kind="ExternalInput")
    rep2_in = nc.dram_tensor("rep2", [16, 128], BF16, kind="ExternalInput")
    out = nc.dram_tensor("out", [B, CO], F32, kind="ExternalOutput")

    with tile.TileContext(nc) as tc:
        with (
            tc.tile_pool(name="stat", bufs=1) as stat,
            tc.tile_pool(name="work", bufs=2) as work,
            tc.tile_pool(name="sm", bufs=1) as smp,
            tc.tile_pool(name="mtp", bufs=4) as mtp,
            tc.tile_pool(name="ytp", bufs=2) as ytp,
            tc.tile_pool(name="dram", bufs=2, space="DRAM") as dram,
            tc.tile_pool(name="ps_mp", bufs=2, space="PSUM") as ps_mp,
            tc.tile_pool(name="ps_yp", bufs=2, space="PSUM") as ps_yp,
            tc.tile_pool(name="ps_ep", bufs=2, space="PSUM") as ps_ep,
            tc.tile_pool(name="ps_sp", bufs=2, space="PSUM") as ps_sp,
        ):
            # ---- static SBUF tensors ----
            XT = stat.tile([128, NT, B], F32R)        # x^T  [(r,i)%128, t, b]
            XBH = stat.tile([128, BH, RI], FP16)      # x    [b%128, bh, (r,i)]
            WS = stat.tile([128, NT, CO], F32R)       # W as lhsT for s-matmul
            WOTH = stat.tile([16, C, RI], FP16)       # W^T as rhs for M-matmul
            REP = stat.tile([128, 8, 128], BF16)      # replicate-by-8 selectors
            REP2 = stat.tile([16, 128], BF16)         # chunk t=8 selector
            IDENT = stat.tile([128, 128], F32)
            IDENTB = stat.tile([128, 128], BF16)
            # chunked + spread over queues: s0's chunk-t matmuls start as
            # soon as their slices land instead of waiting for the whole
            # 1.9MB on one queue.
            for g in range(3):
                ts = slice(3 * g, 3 * g + 3)
                qw = (nc.gpsimd, nc.scalar, nc.gpsimd)[g]
                nc.sync.dma_start(out=XT[:, ts, :], in_=xt_in[:, ts, :])
                qw.dma_start(out=WS[:, ts, :], in_=ws[:, ts, :])
            nc.scalar.dma_start(out=XBH, in_=xbh_in[:])
            nc.gpsimd.dma_start(out=WOTH, in_=woth_in[:])
            nc.scalar.dma_start(out=REP, in_=rep_in[:])
            nc.scalar.dma_start(out=REP2, in_=rep2_in[:])
            make_identity(nc, IDENT[:, :])
            make_identity(nc, IDENTB[:, :])

            # logits b_ij, layout [p=b%128, (bh, r, c)]: c innermost so
            # the softmax max/sum reductions read contiguously
            blog = stat.tile([128, BH, RL, C], F32)
            # previous-iteration softmax shift (see softmax_s1)
            mxs = stat.tile([128, BH, RL], F32)

            # v (squashed capsule outputs), [co, b] layout split by
            # AllReduce group (co 0..96 / 96..160). f32 copies feed the
            # n_iters==1 output transpose; fp16 copies feed the a-phase
            # staging (the a-phase only makes routing logits).
            vA = stat.tile([NA, B], F32)
            vB = stat.tile([NB_, B], F32)
            vhA = stat.tile([NA, B], FP16)
            vhB = stat.tile([NB_, B], FP16)
            # vrt[o, c, b]: a-phase lhsT must start at partition 0,
            # so v-slices are re-staged per capsule via small SBUF DMAs.
            vrt = stat.tile([16, C, B], FP16)
            # en^T staging for the whole s-phase (S1 fills, S2 consumes)
            ET1 = stat.tile([128, C, B], BF16)
            ET2 = stat.tile([16, C, B], BF16)

            def stage_vrt(cs):
                for c in cs:
                    src = (vhA[16 * c:16 * (c + 1), :] if c < CA
                           else vhB[16 * (c - CA):16 * (c - CA + 1), :])
                    qeng = nc.sync if (c % 2 == 0) else nc.scalar
                    qeng.dma_start(out=vrt[:, c, :], in_=src)

            def allreduce_g(writes, grp):
                """One c-group's AllReduce. writes: (ptile, nrows, row0).
                grp 0 covers co 0..96 (c 0..5), grp 1 covers co 96..160.
                Returns (bounce_out, last_drain_tile)."""
                nr = NA if grp == 0 else NB_
                b_in = dram.tile([nr, B], F32, tag=f"ari{grp}")
                b_out = dram.tile([nr, B], F32, tag=f"aro{grp}")
                last_sb = None
                for ptile, nrows, r0 in writes:
                    sb = work.tile([nrows, B], F32, tag=f"sd{grp}_{nrows}")
                    nc.scalar.copy(sb[:, :], ptile[0:nrows, :])
                    nc.sync.dma_start(out=b_in[r0:r0 + nrows, :], in_=sb)
                    last_sb = sb
                if collectives:
                    nc.gpsimd.collective_compute(
                        "AllReduce",
                        mybir.AluOpType.add,
                        replica_groups=[list(range(NCORES))],
                        ins=[b_in[:].opt()],
                        outs=[b_out[:].opt()],
                    )
                else:
                    nc.sync.dma_start(out=b_out[:], in_=b_in[:])
                return b_out, last_sb

            def keepalive(seed, n, nb=128, lhs=None):
                """Chain of dummy accumulating matmuls anchored on `seed`:
                holds the PE HAM activity window open across an engine-idle
                stretch so later matmuls avoid the 1.2GHz cold clock."""
                kp = ps_sp.tile([16, nb], F32, tag="sp")
                li = lhs if lhs is not None else IDENT
                for i in range(n):
                    nc.tensor.matmul(kp, li[0:16, 0:16], seed,
                                     start=(i == 0), stop=(i == n - 1))

            def boost_warm(n):
                """Full-width dummy matmul stream: trips the HAM governor's
                full-rate window while the runtime start barrier blocks the
                first collective anyway, re-phasing the ~133us boost cadence
                so both s-phases land inside granted windows."""
                kb = ps_yp.tile([128, 2 * B], F32, tag="yp")
                for j in range(n):
                    nc.tensor.matmul(kb[:, 0:B], WS[:, 0, 0:128],
                                     XT[:, j % NT, :],
                                     start=(j == 0), stop=(j == n - 1))

            def s0_matmul():
                """s0 = sum_r u_hat -> ONE merged [160, B] AllReduce (there
                is nothing to overlap a split with at startup, and merging
                frees group B ~10us earlier)."""
                p1 = ps_ep.tile([NA, B], F32, tag="ep")
                p2 = ps_sp.tile([NB_, B], F32, tag="sp")
                for t in range(NT):
                    nc.tensor.matmul(p1, WS[:, t, 0:NA], XT[:, t, :],
                                     start=(t == 0), stop=(t == NT - 1))
                    nc.tensor.matmul(p2, WS[:, t, NA:CO], XT[:, t, :],
                                     start=(t == 0), stop=(t == NT - 1))
                boA, sdA = allreduce_g([(p1, NA, 0)], 0)
                boB, _ = allreduce_g([(p2, NB_, 0)], 1)
                boost_warm(60)
                keepalive(sdA[0:16, 0:128], 20)
                return boA, boB

            def squash_chain(sf, sq, ab, den, v, scale):
                """v = s*|s|/(1+s^2) (s*=scale), all on DVE (|s|=max(s,-s)):
                no scalar SQUARE/SQRT (each ACT function switch costs a
                ~1.3us ACT_TABLE_LOAD on the squash critical path)."""
                if scale != 1.0:
                    nc.vector.tensor_scalar_mul(sf, sf, scale)
                nc.vector.tensor_scalar_mul(ab, sf, -1.0)
                nc.vector.tensor_max(ab, ab, sf)
                nc.vector.tensor_mul(sq, sf, sf)
                nc.vector.tensor_scalar_add(den, sq, 1.0)
                nc.vector.reciprocal_approx_fast(den, den)
                nc.vector.tensor_mul(den, ab, den)
                nc.vector.tensor_mul(v, den, sf)

            def squash_g(b_out, scale, grp, stage=True, row0=0):
                """load s [rows,b] from bounce, squash -> v. When staging
                for the a-phase the final mul writes the fp16 copy
                directly (no ACT cast on the staging critical path)."""
                v, vh, nr = (vA, vhA, NA) if grp == 0 else (vB, vhB, NB_)
                s = work.tile([nr, B], F32, tag=f"sq_s{nr}")
                nc.sync.dma_start(out=s, in_=b_out[row0:row0 + nr, :])
                sq = work.tile([nr, B], F32, tag=f"sq_sq{nr}")
                ab = work.tile([nr, B], F32, tag=f"sq_ab{nr}")
                den = work.tile([nr, B], F32, tag=f"sq_den{nr}")
                squash_chain(s[:, :], sq[:, :], ab[:, :], den[:, :],
                             vh[:, :] if stage else v[:, :], scale)

            _apc = [0]

            def a_psum():
                """a-phase M-chunk psum, alternating between the mpsum
                ring and the (s-phase-idle) yp ring: 4 chunk psums in
                flight lets the PE run further ahead of the DVE/ACT/GpSimd
                consumers (the a-phase is latency-bound, not rate-bound)."""
                mpa = ps_mp.tile([128, HA], F32, tag="mpsum")
                return mpa

            def a_blocks(cs, dst):
                """dst[.,bh,c,.] = sum_i x*M, M = v_c @ WoT_c (capsule group).
                M-matmuls in fp16 (half PE cost under the throttled clock).
                dst is blog itself on the first iteration, a fresh ared
                tile afterwards. ~6 blocks multiply on the DVE straight out
                of PSUM; the rest go ACT-copy(fp16) + GpSimd-multiply."""
                for c in cs:
                    for bh in range(BH):
                        lhs = vrt[:, c, bh * 128:(bh + 1) * 128]
                        if bh == 0 and c % 3 != 0:
                            # DVE multiplies straight out of PSUM
                            mt = mtp.tile([128, RI], F32, tag="mtmp")
                            for h in range(3):
                                mp = a_psum()
                                nc.tensor.matmul(
                                    mp[:, :], lhs,
                                    WOTH[:, c, h * HA:(h + 1) * HA],
                                    start=True, stop=True)
                                nc.vector.tensor_mul(
                                    mt[:, h * HA:(h + 1) * HA], mp[:, :],
                                    XBH[:, bh, h * HA:(h + 1) * HA])
                        else:
                            # ACT drains PSUM to fp16, GpSimd multiplies
                            # (keeps the DVE free: it is the binding engine)
                            mt = mtp.tile([128, RI], FP16, tag="mtmp16")
                            ms = mtp.tile([128, RI], FP16, tag="mstage")
                            for h in range(3):
                                mp = a_psum()
                                nc.tensor.matmul(
                                    mp[:, :], lhs,
                                    WOTH[:, c, h * HA:(h + 1) * HA],
                                    start=True, stop=True)
                                nc.scalar.copy(
                                    ms[:, h * HA:(h + 1) * HA], mp[:, :])
                            nc.gpsimd.tensor_mul(mt[:, :], ms[:, :],
                                                 XBH[:, bh, :])
                        tv = mt[:, :].rearrange("p (r i) -> p r i", i=I)
                        nc.vector.tensor_reduce(dst[:, bh, :, c], tv,
                                                axis=mybir.AxisListType.X,
                                                op=mybir.AluOpType.add)

            def blog_update(ar):
                for bh in range(BH):
                    nc.vector.tensor_add(blog[:, bh, :, :], blog[:, bh, :, :],
                                         ar[:, bh, :, :])

            def softmax_s1(bh, sm_tiles):
                """softmax over capsules for one batch half + en^T
                transposes into ET1/ET2. The tiles are allocated once per
                iteration by the caller: per-call allocation would add a
                false WAR dependency serializing the two halves. The
                max-shift is required every round: round-1 logits already
                overflow f32 exp (measured NaN without it), and reusing a
                stale shift either overflows or flushes z to zero."""
                _, e, z, en = sm_tiles
                nc.vector.tensor_reduce(mxs[:, bh, :], blog[:, bh, :, :],
                                        axis=mybir.AxisListType.X,
                                        op=mybir.AluOpType.max)
                nc.vector.tensor_sub(e[:, bh, :, :], blog[:, bh, :, :],
                                     _insert_bcast(mxs[:, bh, :], 2, C))
                nc.scalar.activation(e[:, bh, :, :], e[:, bh, :, :],
                                     mybir.ActivationFunctionType.Exp)
                nc.vector.tensor_reduce(z[:, bh, :], e[:, bh, :, :],
                                        axis=mybir.AxisListType.X,
                                        op=mybir.AluOpType.add)
                nc.vector.reciprocal_approx_fast(z[:, bh, :], z[:, bh, :])
                nc.vector.tensor_mul(en[:, bh, :, :], e[:, bh, :, :],
                                     _insert_bcast(z[:, bh, :], 2, C))
                for c in range(C):
                    ept = ps_ep.tile([128, B + 128], BF16, tag="ep")
                    bs = slice(bh * 128, (bh + 1) * 128)
                    nc.tensor.matmul(ept[:, 0:128], en[:, bh, 0:128, c],
                                     IDENTB[:, :], start=True, stop=True,
                                     is_transpose=True)
                    nc.tensor.matmul(ept[0:16, B:B + 128],
                                     en[:, bh, 128:RL, c],
                                     IDENTB[:, :], start=True, stop=True,
                                     is_transpose=True)
                    nc.scalar.copy(ET1[:, c, bs], ept[:, 0:128])
                    nc.scalar.copy(ET2[:, c, bs], ept[0:16, B:B + 128])

            def s_phase(final=False):
                """S2: rep -> y -> s-matmul streamed back-to-back
                (PE-dense, lands in the HAM boost window).
                Mid-round: AllReduce group A (c0..5) fires right after
                c=5's s-matmul; c6..9 + squash-A + a-blocks overlap the
                collectives; returns (boA, boB).
                Final round: every s_c is transposed on the PE into [b, co]
                orientation as it completes, then ONE AllReduce reduces
                [128, bh*co]; the tail is just squash + output DMA."""
                # S2: rep -> y -> s-matmul, back-to-back on the PE
                writes = []
                fbs = []
                boA = sdA = None
                for c in range(C):
                    ytc = ytp.tile([128, NT, B], F32R, tag="ytc")
                    for pr in range(5):
                        t0 = 2 * pr
                        nn = 1 if pr == 4 else 2
                        yp = ps_yp.tile([128, 2 * B], F32, tag="yp")
                        for k in range(nn):
                            t = t0 + k
                            dst = yp[:, k * B:(k + 1) * B]
                            if t < 8:
                                nc.tensor.matmul(dst, REP[:, t, :],
                                                 ET1[:, c, :],
                                                 start=True, stop=True)
                            else:
                                nc.tensor.matmul(dst, REP2[:, :],
                                                 ET2[:, c, :],
                                                 start=True, stop=True)
                        nc.vector.tensor_mul(
                            ytc[:, t0:t0 + nn, :], yp[:, 0:nn * B],
                            XT[:, t0:t0 + nn, :])

                    sp = ps_sp.tile([16, B], F32, tag="sp")
                    for t in range(NT):
                        nc.tensor.matmul(sp, WS[:, t, c * 16:(c + 1) * 16],
                                         ytc[:, t, :],
                                         start=(t == 0), stop=(t == NT - 1))
                    if final:
                        # transpose s_c -> [b, c*16..] slices of vout now,
                        # while later capsules still compute
                        sb = work.tile([16, B], F32, tag="fsd")
                        nc.scalar.copy(sb[:, :], sp[0:16, :])
                        for bh in range(BH):
                            bs = slice(bh * 128, (bh + 1) * 128)
                            # reuse the (idle) a-phase psum ring: no extra
                            # PSUM banks for the transpose staging
                            tp = ps_mp.tile([128, HA], F32, tag="mpsum")
                            nc.tensor.matmul(tp[:, 0:16], sb[0:16, bs],
                                             IDENT[0:16, 0:16],
                                             start=True, stop=True,
                                             is_transpose=True)
                            nc.scalar.copy(
                                vout[:, bh, c * 16:(c + 1) * 16],
                                tp[:, 0:16])
                        if c == CA - 1 or c == C - 1:
                            # fire this c-group's [b, co] AllReduce now so
                            # the tail squash/DMA pipelines per group
                            g = 0 if c == CA - 1 else 1
                            n0, n1 = (0, NA) if g == 0 else (NA, CO)
                            fi = dram.tile([128, BH, n1 - n0], F32,
                                           tag=f"fbi{g}")
                            fo = dram.tile([128, BH, n1 - n0], F32,
                                           tag=f"fbo{g}")
                            nc.sync.dma_start(out=fi[:, :, :],
                                              in_=vout[:, :, n0:n1])
                            if collectives:
                                nc.gpsimd.collective_compute(
                                    "AllReduce",
                                    mybir.AluOpType.add,
                                    replica_groups=[list(range(NCORES))],
                                    ins=[fi[:].opt()],
                                    outs=[fo[:].opt()],
                                )
                            else:
                                nc.sync.dma_start(out=fo[:], in_=fi[:])
                            fbs.append(fo)
                    else:
                        writes.append((sp, 16, (c - CA if c >= CA else c) * 16))
                        if c == CA - 1:
                            boA, sdA = allreduce_g(writes[0:CA], 0)
                if final:
                    return fbs
                boB, _ = allreduce_g(writes[CA:C], 1)
                keepalive(sdA[0:16, 0:128], 14)
                return boA, boB

            # ---------------- routing ----------------
            vout = stat.tile([128, BH, CO], F32)
            odst = out[:].rearrange("(bh p) co -> p bh co", p=128)
            for _rep in range(reps):
                boA, boB = s0_matmul()
                rowB = 0
                scale = 0.1
                fb = None
                for it in range(1, n_iters):
                    dst = (blog if it == 1
                           else smp.tile([128, BH, RL, C], F32, tag="ared"))
                    squash_g(boA, scale, 0)
                    stage_vrt(range(0, CA))
                    a_blocks(range(0, 2), dst)
                    squash_g(boB, scale, 1, row0=rowB)
                    stage_vrt(range(CA, C))
                    a_blocks(range(2, CA), dst)
                    a_blocks(range(CA, C), dst)
                    if it != 1:
                        blog_update(dst)
                    sm_e = smp.tile([128, BH, RL, C], F32, tag="e")
                    sm_z = smp.tile([128, BH, RL], F32, tag="z")
                    sm_en = smp.tile([128, BH, RL, C], BF16, tag="en")
                    for bh in range(BH):
                        softmax_s1(bh, (None, sm_e, sm_z, sm_en))
                    scale = 1.0
                    rowB = 0
                    if it == n_iters - 1:
                        fb = s_phase(final=True)
                    else:
                        boA, boB = s_phase()
                if fb is not None:
                    # [b, co]-oriented tail per c-group: squash + DMA out
                    for g, fo in enumerate(fb):
                        n0, n1 = (0, NA) if g == 0 else (NA, CO)
                        w = n1 - n0
                        sfin = work.tile([128, BH, w], F32, tag=f"fs{g}")
                        nc.sync.dma_start(out=sfin, in_=fo[:, :, :])
                        sq = work.tile([128, BH, w], F32, tag=f"fsq{g}")
                        ab = work.tile([128, BH, w], F32, tag=f"fab{g}")
                        den = work.tile([128, BH, w], F32, tag=f"fden{g}")
                        vfin = work.tile([128, BH, w], F32, tag=f"fv{g}")
                        squash_chain(sfin[:, :, :], sq[:, :, :],
                                     ab[:, :, :], den[:, :, :],
                                     vfin[:, :, :], scale)
                        og = out[:, n0:n1].rearrange(
                            "(bh p) co -> p bh co", p=128)
                        nc.sync.dma_start(out=og, in_=vfin[:, :, :])
                else:
                    # n_iters == 1: squash s0's [co, b] groups + transpose
                    squash_g(boA, scale, 0, stage=False)
                    for bh in range(BH):
                        bs = slice(bh * 128, (bh + 1) * 128)
                        tp1 = ps_ep.tile([128, NA], F32, tag="ep")
                        nc.tensor.matmul(tp1, vA[:, bs], IDENT[0:NA, 0:NA],
                                         start=True, stop=True,
                                         is_transpose=True)
                        nc.scalar.copy(vout[:, bh, 0:NA], tp1[:, :])
                    squash_g(boB, scale, 1, stage=False, row0=rowB)
                    for bh in range(BH):
                        bs = slice(bh * 128, (bh + 1) * 128)
                        tp2 = ps_ep.tile([128, NB_], F32, tag="ep")
                        nc.tensor.matmul(tp2, vB[:, bs], IDENT[0:NB_, 0:NB_],
                                         start=True, stop=True,
                                         is_transpose=True)
                        nc.scalar.copy(vout[:, bh, NA:CO], tp2[:, :])
                    nc.sync.dma_start(out=odst, in_=vout[:, :, :])

    nc.compile()
    return nc


def prep_inputs(x: np.ndarray, W: np.ndarray):
    """Host-side layout prep. Returns per-core input dicts."""
    W = W[0]  # [R, C, O, I]
    # replicate-by-8 selector masks (shared across cores)
    from ml_dtypes import bfloat16
    rep = np.zeros((128, 8, 128), dtype=bfloat16)
    for t in range(8):
        for j in range(128):
            rep[16 * t + j // 8, t, j] = 1.0
    rep2 = np.zeros((16, 128), dtype=bfloat16)
    for j in range(128):
        rep2[j // 8, j] = 1.0
    in_maps = []
    for k in range(NCORES):
        rs = slice(k * RL, (k + 1) * RL)
        xk = np.ascontiguousarray(x[:, rs, :])      # [B, RL, I]
        wk = np.ascontiguousarray(W[rs])            # [RL, C, O, I]
        xt = np.transpose(xk, (1, 2, 0)).reshape(NT, 128, B)
        xt = np.transpose(xt, (1, 0, 2))            # [128, NT, B]
        xb = xk.reshape(BH, 128, RI)
        xb = np.transpose(xb, (1, 0, 2))            # [128, BH, RI]
        # ws[p, t, c*16+o] = W[16t + p//8, c, o, p%8]
        wsk = np.transpose(wk.reshape(NT, 16, C, O, I), (0, 1, 4, 2, 3))
        wsk = wsk.reshape(NT, 128, CO)
        wsk = np.transpose(wsk, (1, 0, 2))          # [128, NT, CO]
        # wot[o, c, r*8+i] = W[r, c, o, i]
        wotk = np.transpose(wk, (2, 1, 0, 3)).reshape(O, C, RI)
        f32 = np.float32
        in_maps.append({
            "xt": np.ascontiguousarray(xt).astype(f32),
            "xbh": np.ascontiguousarray(xb).astype(np.float16),
            "ws": np.ascontiguousarray(wsk).astype(f32),
            "woth": np.ascontiguousarray(wotk).astype(np.float16),
            "rep": rep,
            "rep2": rep2,
        })
    return in_maps


_CACHE = {}


def _get_nc(n_iters: int):
    if n_iters not in _CACHE:
        _CACHE[n_iters] = build_kernel(n_iters)
    return _CACHE[n_iters]


def kernel(x, W, num_iterations, _trace=False):
    n = int(num_iterations)
    assert n >= 1
    nc = _get_nc(n)
    in_maps = prep_inputs(np.asarray(x, dtype=np.float32),
                          np.asarray(W, dtype=np.float32))
    res = run_bass_kernel_spmd(nc, in_maps, list(range(NCORES)),
                               trace=_trace)
    v = res.results[0]["out"].reshape(B, C, O, 1).astype(np.float32)
    kernel.last_results = res
    return v


# revision 53
# speedup vs baseline: 1.1291x; 1.0355x over previous
"""DigitCaps dynamic-routing kernel for 8 TRN2 NeuronCores.

Problem (hardcoded): x [256,1152,8] f32, W [1,1152,10,16,8] f32, 3 routing
iterations -> v [256,10,16,1] f32.

Strategy: shard the R=1152 routes 8-ways (144 per core), keep the full batch
B=256 on every core. u_hat is never materialized; each routing iteration
streams W through the TensorEngine:
  s_c[o,b]   = sum_{(r,i)} Ws_c[(r,i),o] * (en_c[r,b] * x[(r,i),b])   (PE)
  (AllReduce s over the 8 R-shards, squash -> v on every core)
  M_c[b,(r,i)] = sum_o v_c[b,o] * WoT_c[o,(r,i)]                      (PE)
  a_c[b,r]   = sum_i x[b,(r,i)] * M_c[b,(r,i)]                        (DVE)

Optimizations over the 373us baseline (now ~350us; ~265us of that is
kernel work after the first AllReduce - the rest is a fixed runtime
start barrier of 21+[30..55]us launch skew plus the first collective):
  - input DMAs split per-chunk across 3 engine queues so s0's first
    matmul starts as soon as chunk 0 of XT/WS lands (25us -> 13.5us);
  - all AllReduces are split c0..5/c6..9; the per-iteration group A is
    triggered inside the s-phase right after c=5's s-matmul, so c6..9 +
    squash-A + a-blocks cover both collectives (4/6 grouping measured
    worse: the bigger B collective lands later and slows the fin tail);
  - a "boost-warm" chain of full-width dummy matmuls after s0: the HAM
    utilization governor grants one 81920-cycle full-rate window per
    ~133us; burning a window inside the start barrier re-phases the
    governor so BOTH s-phases land in granted windows (s-matmuls at
    2 instead of 4 cycles/column);
  - the s-phase is split S1/S2: S1 (softmax + all en-transposes,
    PE-light) runs first, S2 (rep + s-matmuls, PE-dense) streams
    back-to-back into the governor's window;
  - blog is laid out [b, bh, r, c] with the capsule axis INNERMOST so
    the softmax max/sum reductions read contiguously (strided c-reads
    cost 0.56 vs 0.9 elem/ns/partition on the DVE);
  - the final s-phase transposes each s_c on the PE as it completes and
    fires per-group [b, co]-oriented AllReduces; the post-collective
    tail is just squash + the output DMA (no transposes after);
  - the a-phase runs in fp16 (WoT, x-copy, v-staging, M-matmuls, x*M
    products on the GpSimd path). Numerically safe: the a-phase only
    produces ROUTING LOGITS; the output path s = Ws@(en*x) stays f32r.
    fp16 M-matmuls halve the PE time under the throttled clock. NOTE:
    fp16/bf16 do NOT speed up DVE elementwise/reduce ops on TRN2
    (measured 1 elem/cycle/lane regardless of dtype);
  - squash runs entirely on DVE (|s| = max(s,-s); abs_max is not
    supported by codegen): the scalar-engine SQUARE/SQRT variants each
    cost a ~1.3us ACT_TABLE_LOAD on the post-collective critical path,
    and the staging copy writes fp16 v directly;
  - en = e*z normalization batched into one DVE op per batch-half;
  - softmax max-shift retained (logits reach ~+-70/round; exp would
    overflow); broadcasts use step-0 APs; smp softmax tiles are
    allocated once per iteration (per-call allocation adds a false WAR
    dependency that serializes the batch halves);
  - dummy keepalive matmul chains cover the collective windows.
Known dead ends (measured): bf16 collective payloads (5e-2 error AND a
slow CC path), bh-outer a-phase with overlapped softmax (DVE work is
the wall; reordering does not cut it), merged both-halves reduces
(gates on the slow GpSimd path), GpSimd anything-but-mult (Pool engine
rejects max/tensor_scalar, cannot read PSUM). W/x stay f32(r) on the
s-path: routing is chaotic under bf16 W/x rounding of the s
contraction (measured 5e-2 vs 2e-2 tolerance).
"""

import sys

if "/opt/trn_rl_repo" not in sys.path:
    sys.path.insert(0, "/opt/trn_rl_repo")

import numpy as np

import concourse.bass as bass
import concourse.tile as tile
from concourse import bacc, mybir
from concourse.bass_utils import run_bass_kernel_spmd
from concourse.masks import make_identity

F32 = mybir.dt.float32
F32R = mybir.dt.float32r
BF16 = mybir.dt.bfloat16
FP16 = mybir.dt.float16

NCORES = 8
B, R, C, O, I = 256, 1152, 10, 16, 8
RL = R // NCORES          # 144 routes per core
RI = RL * I               # 1152 (r,i) rows per core
NT = RI // 128            # 9 K-chunks of 128
CO = C * O                # 160
BH = B // 128             # 2 batch half-tiles
HA = RI // 3              # 384-wide a-phase chunks
CA, CB = 6, 4             # AllReduce capsule groups c0..5 / c6..9
NA, NB_ = CA * O, CB * O  # 96 / 64 bounce rows

AP = bass.AP


def _insert_bcast(base, pos, count):
    """Insert a step-0 (broadcast) free dim into an existing AP at index pos."""
    dims = list(base.ap)
    dims.insert(pos, [0, count])
    return AP(tensor=base.tensor, offset=base.offset, ap=dims)


def build_kernel(n_iters: int, reps: int = 1, collectives: bool = True):
    nc = bacc.Bacc("TRN2", target_bir_lowering=False, debug=False,
                   num_devices=NCORES)

    xt_in = nc.dram_tensor("xt", [128, NT, B], F32R, kind="ExternalInput")
    xbh_in = nc.dram_tensor("xbh", [128, BH, RI], FP16, kind="ExternalInput")
    ws = nc.dram_tensor("ws", [128, NT, CO], F32R, kind="ExternalInput")
    woth_in = nc.dram_tensor("woth", [16, C, RI], FP16, kind="ExternalInput")
    rep_in = nc.dram_tensor("rep", [128, 8, 128], BF16, kind="ExternalInput")
    rep2_in = nc.dram_tensor("rep2", [16, 128], BF16, kind="ExternalInput")
    out = nc.dram_tensor("out", [B, CO], F32, kind="ExternalOutput")

    with tile.TileContext(nc) as tc:
        with (
            tc.tile_pool(name="stat", bufs=1) as stat,
            tc.tile_pool(name="work", bufs=2) as work,
            tc.tile_pool(name="sm", bufs=1) as smp,
            tc.tile_pool(name="mtp", bufs=4) as mtp,
            tc.tile_pool(name="ytp", bufs=2) as ytp,
            tc.tile_pool(name="dram", bufs=2, space="DRAM") as dram,
            tc.tile_pool(name="ps_mp", bufs=2, space="PSUM") as ps_mp,
            tc.tile_pool(name="ps_yp", bufs=2, space="PSUM") as ps_yp,
            tc.tile_pool(name="ps_ep", bufs=2, space="PSUM") as ps_ep,
            tc.tile_pool(name="ps_sp", bufs=2, space="PSUM") as ps_sp,
        ):
            # ---- static SBUF tensors ----
            XT = stat.tile([128, NT, B], F32R)        # x^T  [(r,i)%128, t, b]
            XBH = stat.tile([128, BH, RI], FP16)      # x    [b%128, bh, (r,i)]
            WS = stat.tile([128, NT, CO], F32R)       # W as lhsT for s-matmul
            WOTH = stat.tile([16, C, RI], FP16)       # W^T as rhs for M-matmul
            REP = stat.tile([128, 8, 128], BF16)      # replicate-by-8 selectors
            REP2 = stat.tile([16, 128], BF16)         # chunk t=8 selector
            IDENT = stat.tile([128, 128], F32)
            IDENTB = stat.tile([128, 128], BF16)
            # chunked + spread over queues: s0's chunk-t matmuls start as
            # soon as their slices land instead of waiting for the whole
            # 1.9MB on one queue.
            for g in range(3):
                ts = slice(3 * g, 3 * g + 3)
                qw = (nc.gpsimd, nc.scalar, nc.gpsimd)[g]
                nc.sync.dma_start(out=XT[:, ts, :], in_=xt_in[:, ts, :])
                qw.dma_start(out=WS[:, ts, :], in_=ws[:, ts, :])
            nc.scalar.dma_start(out=XBH, in_=xbh_in[:])
            nc.gpsimd.dma_start(out=WOTH, in_=woth_in[:])
            nc.scalar.dma_start(out=REP, in_=rep_in[:])
            nc.scalar.dma_start(out=REP2, in_=rep2_in[:])
            make_identity(nc, IDENT[:, :])
            make_identity(nc, IDENTB[:, :])

            # logits b_ij, layout [p=b%128, (bh, r, c)]: c innermost so
            # the softmax max/sum reductions read contiguously
            blog = stat.tile([128, BH, RL, C], F32)
            # previous-iteration softmax shift (see softmax_s1)
            mxs = stat.tile([128, BH, RL], F32)

            # v (squashed capsule outputs), [co, b] layout split by
            # AllReduce group (co 0..96 / 96..160). f32 copies feed the
            # n_iters==1 output transpose; fp16 copies feed the a-phase
            # staging (the a-phase only makes routing logits).
            vA = stat.tile([NA, B], F32)
            vB = stat.tile([NB_, B], F32)
            vhA = stat.tile([NA, B], FP16)
            vhB = stat.tile([NB_, B], FP16)
            # vrt[o, c, b]: a-phase lhsT must start at partition 0,
            # so v-slices are re-staged per capsule via small SBUF DMAs.
            vrt = stat.tile([16, C, B], FP16)
            # en^T staging for the whole s-phase (S1 fills, S2 consumes)
            ET1 = stat.tile([128, C, B], BF16)
            ET2 = stat.tile([16, C, B], BF16)

            def vh_slice(c):
                return (vhA[16 * c:16 * (c + 1), :] if c < CA
                        else vhB[16 * (c - CA):16 * (c - CA + 1), :])

            def stage_vrt(cs):
                # c=0 / c=CA sit at partition 0 of vhA/vhB and feed the
                # M-matmul lhsT directly (matmul requires lhsT and rhs to
                # share a base partition, so only offset-0 slices qualify);
                # all other capsules need the re-staging DMA. This takes
                # the stage DMA off the post-squash critical ramp for the
                # first block of each group.
                for c in cs:
                    if c not in (0, CA):
                        qeng = nc.sync if (c % 2 == 0) else nc.scalar
                        qeng.dma_start(out=vrt[:, c, :], in_=vh_slice(c))

            def allreduce_g(writes, grp):
                """One c-group's AllReduce. writes: (ptile, nrows, row0).
                grp 0 covers co 0..96 (c 0..5), grp 1 covers co 96..160.
                Returns (bounce_out, last_drain_tile)."""
                nr = NA if grp == 0 else NB_
                b_in = dram.tile([nr, B], F32, tag=f"ari{grp}")
                b_out = dram.tile([nr, B], F32, tag=f"aro{grp}")
                last_sb = None
                for ptile, nrows, r0 in writes:
                    sb = work.tile([nrows, B], F32, tag=f"sd{grp}_{nrows}")
                    nc.scalar.copy(sb[:, :], ptile[0:nrows, :])
                    nc.sync.dma_start(out=b_in[r0:r0 + nrows, :], in_=sb)
                    last_sb = sb
                if collectives:
                    nc.gpsimd.collective_compute(
                        "AllReduce",
                        mybir.AluOpType.add,
                        replica_groups=[list(range(NCORES))],
                        ins=[b_in[:].opt()],
                        outs=[b_out[:].opt()],
                    )
                else:
                    nc.sync.dma_start(out=b_out[:], in_=b_in[:])
                return b_out, last_sb

            def keepalive(seed, n, nb=128, lhs=None):
                """Chain of dummy accumulating matmuls anchored on `seed`:
                holds the PE HAM activity window open across an engine-idle
                stretch so later matmuls avoid the 1.2GHz cold clock."""
                kp = ps_sp.tile([16, nb], F32, tag="sp")
                li = lhs if lhs is not None else IDENT
                for i in range(n):
                    nc.tensor.matmul(kp, li[0:16, 0:16], seed,
                                     start=(i == 0), stop=(i == n - 1))

            def boost_warm(n):
                """Full-width dummy matmul stream: trips the HAM governor's
                full-rate window while the runtime start barrier blocks the
                first collective anyway, re-phasing the ~133us boost cadence
                so both s-phases land inside granted windows."""
                kb = ps_yp.tile([128, 2 * B], F32, tag="yp")
                for j in range(n):
                    nc.tensor.matmul(kb[:, 0:B], WS[:, 0, 0:128],
                                     XT[:, j % NT, :],
                                     start=(j == 0), stop=(j == n - 1))

            def s0_matmul():
                """s0 = sum_r u_hat -> ONE merged [160, B] AllReduce (there
                is nothing to overlap a split with at startup, and merging
                frees group B ~10us earlier)."""
                p1 = ps_ep.tile([NA, B], F32, tag="ep")
                p2 = ps_sp.tile([NB_, B], F32, tag="sp")
                for t in range(NT):
                    nc.tensor.matmul(p1, WS[:, t, 0:NA], XT[:, t, :],
                                     start=(t == 0), stop=(t == NT - 1))
                    nc.tensor.matmul(p2, WS[:, t, NA:CO], XT[:, t, :],
                                     start=(t == 0), stop=(t == NT - 1))
                boA, sdA = allreduce_g([(p1, NA, 0)], 0)
                boB, _ = allreduce_g([(p2, NB_, 0)], 1)
                boost_warm(60)
                keepalive(sdA[0:16, 0:128], 20)
                return boA, boB

            def squash_chain(sf, sq, ab, den, v, scale):
                """v = s*|s|/(1+s^2) (s*=scale), all on DVE (|s|=max(s,-s)):
                no scalar SQUARE/SQRT (each ACT function switch costs a
                ~1.3us ACT_TABLE_LOAD on the squash critical path)."""
                if scale != 1.0:
                    nc.vector.tensor_scalar_mul(sf, sf, scale)
                nc.vector.tensor_scalar_mul(ab, sf, -1.0)
                nc.vector.tensor_max(ab, ab, sf)
                nc.vector.tensor_mul(sq, sf, sf)
                nc.vector.tensor_scalar_add(den, sq, 1.0)
                nc.vector.reciprocal_approx_fast(den, den)
                nc.vector.tensor_mul(den, ab, den)
                nc.vector.tensor_mul(v, den, sf)

            def squash_g(b_out, scale, grp, stage=True, row0=0):
                """load s [rows,b] from bounce, squash -> v. When staging
                for the a-phase the final mul writes the fp16 copy
                directly (no ACT cast on the staging critical path)."""
                v, vh, nr = (vA, vhA, NA) if grp == 0 else (vB, vhB, NB_)
                s = work.tile([nr, B], F32, tag=f"sq_s{nr}")
                nc.sync.dma_start(out=s, in_=b_out[row0:row0 + nr, :])
                sq = work.tile([nr, B], F32, tag=f"sq_sq{nr}")
                ab = work.tile([nr, B], F32, tag=f"sq_ab{nr}")
                den = work.tile([nr, B], F32, tag=f"sq_den{nr}")
                squash_chain(s[:, :], sq[:, :], ab[:, :], den[:, :],
                             vh[:, :] if stage else v[:, :], scale)

            _apc = [0]

            def a_psum():
                """a-phase M-chunk psum, alternating between the mpsum
                ring and the (s-phase-idle) yp ring: 4 chunk psums in
                flight lets the PE run further ahead of the DVE/ACT/GpSimd
                consumers (the a-phase is latency-bound, not rate-bound)."""
                mpa = ps_mp.tile([128, HA], F32, tag="mpsum")
                return mpa

            def a_blocks(cs, dst):
                """dst[.,bh,c,.] = sum_i x*M, M = v_c @ WoT_c (capsule group).
                M-matmuls in fp16 (half PE cost under the throttled clock).
                dst is blog itself on the first iteration, a fresh ared
                tile afterwards. ~6 blocks multiply on the DVE straight out
                of PSUM; the rest go ACT-copy(fp16) + GpSimd-multiply."""
                for c in cs:
                    for bh in range(BH):
                        bs = slice(bh * 128, (bh + 1) * 128)
                        lhs = (vh_slice(c)[:, bs] if c in (0, CA)
                               else vrt[:, c, bs])
                        if bh == 0 and c % 3 != 0:
                            # DVE multiplies straight out of PSUM
                            mt = mtp.tile([128, RI], F32, tag="mtmp")
                            for h in range(3):
                                mp = a_psum()
                                nc.tensor.matmul(
                                    mp[:, :], lhs,
                                    WOTH[:, c, h * HA:(h + 1) * HA],
                                    start=True, stop=True)
                                nc.vector.tensor_mul(
                                    mt[:, h * HA:(h + 1) * HA], mp[:, :],
                                    XBH[:, bh, h * HA:(h + 1) * HA])
                        else:
                            # ACT drains PSUM to fp16, GpSimd multiplies
                            # (keeps the DVE free: it is the binding engine)
                            mt = mtp.tile([128, RI], FP16, tag="mtmp16")
                            ms = mtp.tile([128, RI], FP16, tag="mstage")
                            for h in range(3):
                                mp = a_psum()
                                nc.tensor.matmul(
                                    mp[:, :], lhs,
                                    WOTH[:, c, h * HA:(h + 1) * HA],
                                    start=True, stop=True)
                                nc.scalar.copy(
                                    ms[:, h * HA:(h + 1) * HA], mp[:, :])
                            nc.gpsimd.tensor_mul(mt[:, :], ms[:, :],
                                                 XBH[:, bh, :])
                        tv = mt[:, :].rearrange("p (r i) -> p r i", i=I)
                        nc.vector.tensor_reduce(dst[:, bh, :, c], tv,
                                                axis=mybir.AxisListType.X,
                                                op=mybir.AluOpType.add)

            def blog_update(ar):
                for bh in range(BH):
                    nc.vector.tensor_add(blog[:, bh, :, :], blog[:, bh, :, :],
                                         ar[:, bh, :, :])

            def softmax_s1(bh, sm_tiles):
                """softmax over capsules for one batch half + en^T
                transposes into ET1/ET2. The tiles are allocated once per
                iteration by the caller: per-call allocation would add a
                false WAR dependency serializing the two halves. The
                max-shift is required every round: round-1 logits already
                overflow f32 exp (measured NaN without it), and reusing a
                stale shift either overflows or flushes z to zero."""
                _, e, z, en = sm_tiles
                nc.vector.tensor_reduce(mxs[:, bh, :], blog[:, bh, :, :],
                                        axis=mybir.AxisListType.X,
                                        op=mybir.AluOpType.max)
                nc.vector.tensor_sub(e[:, bh, :, :], blog[:, bh, :, :],
                                     _insert_bcast(mxs[:, bh, :], 2, C))
                nc.scalar.activation(e[:, bh, :, :], e[:, bh, :, :],
                                     mybir.ActivationFunctionType.Exp)
                nc.vector.tensor_reduce(z[:, bh, :], e[:, bh, :, :],
                                        axis=mybir.AxisListType.X,
                                        op=mybir.AluOpType.add)
                nc.vector.reciprocal_approx_fast(z[:, bh, :], z[:, bh, :])
                nc.vector.tensor_mul(en[:, bh, :, :], e[:, bh, :, :],
                                     _insert_bcast(z[:, bh, :], 2, C))
                for c in range(C):
                    ept = ps_ep.tile([128, B + 128], BF16, tag="ep")
                    bs = slice(bh * 128, (bh + 1) * 128)
                    nc.tensor.matmul(ept[:, 0:128], en[:, bh, 0:128, c],
                                     IDENTB[:, :], start=True, stop=True,
                                     is_transpose=True)
                    nc.tensor.matmul(ept[0:16, B:B + 128],
                                     en[:, bh, 128:RL, c],
                                     IDENTB[:, :], start=True, stop=True,
                                     is_transpose=True)
                    nc.scalar.copy(ET1[:, c, bs], ept[:, 0:128])
                    nc.scalar.copy(ET2[:, c, bs], ept[0:16, B:B + 128])

            def s_phase(final=False):
                """S2: rep -> y -> s-matmul streamed back-to-back
                (PE-dense, lands in the HAM boost window).
                Mid-round: AllReduce group A (c0..5) fires right after
                c=5's s-matmul; c6..9 + squash-A + a-blocks overlap the
                collectives; returns (boA, boB).
                Final round: every s_c is transposed on the PE into [b, co]
                orientation as it completes, then ONE AllReduce reduces
                [128, bh*co]; the tail is just squash + output DMA."""
                # S2: rep -> y -> s-matmul, back-to-back on the PE
                writes = []
                fbs = []
                boA = sdA = None
                for c in range(C):
                    ytc = ytp.tile([128, NT, B], F32R, tag="ytc")
                    for pr in range(5):
                        t0 = 2 * pr
                        nn = 1 if pr == 4 else 2
                        yp = ps_yp.tile([128, 2 * B], F32, tag="yp")
                        for k in range(nn):
                            t = t0 + k
                            dst = yp[:, k * B:(k + 1) * B]
                            if t < 8:
                                nc.tensor.matmul(dst, REP[:, t, :],
                                                 ET1[:, c, :],
                                                 start=True, stop=True)
                            else:
                                nc.tensor.matmul(dst, REP2[:, :],
                                                 ET2[:, c, :],
                                                 start=True, stop=True)
                        nc.vector.tensor_mul(
                            ytc[:, t0:t0 + nn, :], yp[:, 0:nn * B],
                            XT[:, t0:t0 + nn, :])

                    sp = ps_sp.tile([16, B], F32, tag="sp")
                    for t in range(NT):
                        nc.tensor.matmul(sp, WS[:, t, c * 16:(c + 1) * 16],
                                         ytc[:, t, :],
                                         start=(t == 0), stop=(t == NT - 1))
                    if final:
                        # transpose s_c -> [b, c*16..] slices of vout now,
                        # while later capsules still compute
                        sb = work.tile([16, B], F32, tag="fsd")
                        nc.scalar.copy(sb[:, :], sp[0:16, :])
                        for bh in range(BH):
                            bs = slice(bh * 128, (bh + 1) * 128)
                            # reuse the (idle) a-phase psum ring: no extra
                            # PSUM banks for the transpose staging
                            tp = ps_mp.tile([128, HA], F32, tag="mpsum")
                            nc.tensor.matmul(tp[:, 0:16], sb[0:16, bs],
                                             IDENT[0:16, 0:16],
                                             start=True, stop=True,
                                             is_transpose=True)
                            nc.scalar.copy(
                                vout[:, bh, c * 16:(c + 1) * 16],
                                tp[:, 0:16])
                        if c == CA - 1 or c == C - 1:
                            # fire this c-group's [b, co] AllReduce now so
                            # the tail squash/DMA pipelines per group
                            g = 0 if c == CA - 1 else 1
                            n0, n1 = (0, NA) if g == 0 else (NA, CO)
                            fi = dram.tile([128, BH, n1 - n0], F32,
                                           tag=f"fbi{g}")
                            fo = dram.tile([128, BH, n1 - n0], F32,
                                           tag=f"fbo{g}")
                            nc.sync.dma_start(out=fi[:, :, :],
                                              in_=vout[:, :, n0:n1])
                            if collectives:
                                nc.gpsimd.collective_compute(
                                    "AllReduce",
                                    mybir.AluOpType.add,
                                    replica_groups=[list(range(NCORES))],
                                    ins=[fi[:].opt()],
                                    outs=[fo[:].opt()],
                                )
                            else:
                                nc.sync.dma_start(out=fo[:], in_=fi[:])
                            fbs.append(fo)
                    else:
                        writes.append((sp, 16, (c - CA if c >= CA else c) * 16))
                        if c == CA - 1:
                            boA, sdA = allreduce_g(writes[0:CA], 0)
                if final:
                    return fbs
                boB, _ = allreduce_g(writes[CA:C], 1)
                keepalive(sdA[0:16, 0:128], 14)
                return boA, boB

            # ---------------- routing ----------------
            vout = stat.tile([128, BH, CO], F32)
            odst = out[:].rearrange("(bh p) co -> p bh co", p=128)
            for _rep in range(reps):
                boA, boB = s0_matmul()
                rowB = 0
                scale = 0.1
                fb = None
                for it in range(1, n_iters):
                    dst = (blog if it == 1
                           else smp.tile([128, BH, RL, C], F32, tag="ared"))
                    squash_g(boA, scale, 0)
                    stage_vrt(range(0, CA))
                    a_blocks(range(0, 2), dst)
                    squash_g(boB, scale, 1, row0=rowB)
                    stage_vrt(range(CA, C))
                    a_blocks(range(2, CA), dst)
                    a_blocks(range(CA, C), dst)
                    if it != 1:
                        blog_update(dst)
                    sm_e = smp.tile([128, BH, RL, C], F32, tag="e")
                    sm_z = smp.tile([128, BH, RL], F32, tag="z")
                    sm_en = smp.tile([128, BH, RL, C], BF16, tag="en")
                    for bh in range(BH):
                        softmax_s1(bh, (None, sm_e, sm_z, sm_en))
                    scale = 1.0
                    rowB = 0
                    if it == n_iters - 1:
                        fb = s_phase(final=True)
                    else:
                        boA, boB = s_phase()
                if fb is not None:
                    # [b, co]-oriented tail per c-group: squash + DMA out
                    for g, fo in enumerate(fb):
                        n0, n1 = (0, NA) if g == 0 else (NA, CO)
                        w = n1 - n0
                        sfin = work.tile([128, BH, w], F32, tag=f"fs{g}")
                        nc.sync.dma_start(out=sfin, in_=fo[:, :, :])
                        sq = work.tile([128, BH, w], F32, tag=f"fsq{g}")
                        ab = work.tile([128, BH, w], F32, tag=f"fab{g}")
                        den = work.tile([128, BH, w], F32, tag=f"fden{g}")
                        vfin = work.tile([128, BH, w], F32, tag=f"fv{g}")
                        squash_chain(sfin[:, :, :], sq[:, :, :],
                                     ab[:, :, :], den[:, :, :],
                                     vfin[:, :, :], scale)
                        og = out[:, n0:n1].rearrange(
                            "(bh p) co -> p bh co", p=128)
                        nc.sync.dma_start(out=og, in_=vfin[:, :, :])
                else:
                    # n_iters == 1: squash s0's [co, b] groups + transpose
                    squash_g(boA, scale, 0, stage=False)
                    for bh in range(BH):
                        bs = slice(bh * 128, (bh + 1) * 128)
                        tp1 = ps_ep.tile([128, NA], F32, tag="ep")
                        nc.tensor.matmul(tp1, vA[:, bs], IDENT[0:NA, 0:NA],
                                         start=True, stop=True,
                                         is_transpose=True)
                        nc.scalar.copy(vout[:, bh, 0:NA], tp1[:, :])
                    squash_g(boB, scale, 1, stage=False, row0=rowB)
                    for bh in range(BH):
                        bs = slice(bh * 128, (bh + 1) * 128)
                        tp2 = ps_ep.tile([128, NB_], F32, tag="ep")
                        nc.tensor.matmul(tp2, vB[:, bs], IDENT[0:NB_, 0:NB_],
                                         start=True, stop=True,
                                         is_transpose=True)
                        nc.scalar.copy(vout[:, bh, NA:CO], tp2[:, :])
                    nc.sync.dma_start(out=odst, in_=vout[:, :, :])

    nc.compile()
    return nc


def prep_inputs(x: np.ndarray, W: np.ndarray):
    """Host-side layout prep. Returns per-core input dicts."""
    W = W[0]  # [R, C, O, I]
    # replicate-by-8 selector masks (shared across cores)
    from ml_dtypes import bfloat16
    rep = np.zeros((128, 8, 128), dtype=bfloat16)
    for t in range(8):
        for j in range(128):
            rep[16 * t + j // 8, t, j] = 1.0
    rep2 = np.zeros((16, 128), dtype=bfloat16)
    for j in range(128):
        rep2[j // 8, j] = 1.0
    in_maps = []
    for k in range(NCORES):
        rs = slice(k * RL, (k + 1) * RL)
        xk = np.ascontiguousarray(x[:, rs, :])      # [B, RL, I]
        wk = np.ascontiguousarray(W[rs])            # [RL, C, O, I]
        xt = np.transpose(xk, (1, 2, 0)).reshape(NT, 128, B)
        xt = np.transpose(xt, (1, 0, 2))            # [128, NT, B]
        xb = xk.reshape(BH, 128, RI)
        xb = np.transpose(xb, (1, 0, 2))            # [128, BH, RI]
        # ws[p, t, c*16+o] = W[16t + p//8, c, o, p%8]
        wsk = np.transpose(wk.reshape(NT, 16, C, O, I), (0, 1, 4, 2, 3))
        wsk = wsk.reshape(NT, 128, CO)
        wsk = np.transpose(wsk, (1, 0, 2))          # [128, NT, CO]
        # wot[o, c, r*8+i] = W[r, c, o, i]
        wotk = np.transpose(wk, (2, 1, 0, 3)).reshape(O, C, RI)
        f32 = np.float32
        in_maps.append({
            "xt": np.ascontiguousarray(xt).astype(f32),
            "xbh": np.ascontiguousarray(xb).astype(np.float16),
            "ws": np.ascontiguousarray(wsk).astype(f32),
            "woth": np.ascontiguousarray(wotk).astype(np.float16),
            "rep": rep,
            "rep2": rep2,
        })
    return in_maps


_CACHE = {}


def _get_nc(n_iters: int):
    if n_iters not in _CACHE:
        _CACHE[n_iters] = build_kernel(n_iters)
    return _CACHE[n_iters]


def kernel(x, W, num_iterations, _trace=False):
    n = int(num_iterations)
    assert n >= 1
    nc = _get_nc(n)
    in_maps = prep_inputs(np.asarray(x, dtype=np.float32),
                          np.asarray(W, dtype=np.float32))
    res = run_bass_kernel_spmd(nc, in_maps, list(range(NCORES)),
                               trace=_trace)
    v = res.results[0]["out"].reshape(B, C, O, 1).astype(np.float32)
    kernel.last_results = res
    return v
